# revision 45
# baseline (speedup 1.0000x reference)
"""Trainium2 Bass kernel for nn_AttGRU (B=16, S=64, N=2048, E=256) on 8 NeuronCores.

Default variant "v7" (legacy variants kept below for reference):
  - scores via the K=65 trick: sT[j,i] = xh_j^T M xh_i with M = 65x65 host-
    precomputed from Wq/Wk/bq/bk; scores matmuls in bf16.
  - phase A/B (attention + AGG precompute), i-sharded 8 ways, pipelined in
    batch-quarters: scores(q) overlaps AllReduce(q-1) and AGG(q-1), so the 4
    D-AllReduces are off the critical path. Mask applied as a bf16 0/1
    multiply after exp (no mask matmuls); D via one DVE 3D reduce per batch.
  - phase C (GRU, 64 sequential steps), i-sharded: per step one gate GEMM
    (bf16, 2-way PE column tiling, r|z segment then n segment so the rz gate
    math overlaps the n GEMM), gates on DVE/ACT (fp32 where a PSUM operand
    sets the rate, bf16 elsewhere), PE transposes of the h slice, and an 8KB
    bf16 AllGather with a p-major payload layout chosen so the SBUF->DRAM
    bounce and the reload are contiguous-per-partition DMAs split across the
    sync/scalar/gpsimd queues.
  - numerics: bf16 scores/E/AGG/h, fp32 PSUM accumulation everywhere;
    rel err vs fp32 reference ~8.5e-3 (tolerance 2e-2), deterministic.
  - phase C step details: h update computed as (1-z)*n + z*h with zc=1-z and
    z*h prepared during ACT/DVE slack so only two DVE ops follow tanh; reload
    as four stride-4 rank-pair DMAs so the first GEMM k-pair waits on a
    single semaphore; the 8KB SBUF->DRAM bounce split across two queues.
  - phase A/B AGG: the 16 per-jt x*Dinv scalar-muls are one DVE multiply
    with a stride-0 broadcast operand (dinv[:, jt, None].to_broadcast), since
    DVE tensor ops accept broadcast SBUF APs directly.
  - measured (NTFF): ~1.52 ms total = ~0.24 ms phase A/B + ~1.28 ms
    phase C (64 steps x ~20 us; step = reload 2.9 + GEMM 4.2 + gates 3.6 +
    transpose/copy 0.6 + bounce+trigger 3.2 + AllGather ~5.2).
    Baseline "abc" variant: 3.30 ms by the same measurement.
    Remaining A/B cap is DVE element throughput (masks 3.3us + D-reduce
    4.4us per batch); remaining C cap is collective latency (~11.3us/step).
  - negative results (all measured): 4-way PE column tiling (quadrant 3 is
    HW-broken), single-stream GEMM (2-way col tiling IS concurrent), any
    tensor ops on the Pool engine (phase C gates AND A/B masks/xd both
    regressed ~100-150us), HAM warm-up matmuls, fp8 h transport (3.1e-2 err:
    HW fp8 conversion loses more than ml_dtypes RNE predicts -- also why
    variant "v8" fp8 DoubleRow, ~1.54ms, fails at 4e-2; both unused),
    s_psum bufs 3->4 (noise).
"""

import sys

for _p in ("/opt/trn_rl_repo", "/root/.axon_site/_ro/trn_rl_repo"):
    if _p not in sys.path:
        sys.path.append(_p)

import numpy as np
from contextlib import ExitStack

import concourse.bacc as bacc
import concourse.tile as tile
import concourse.mybir as mybir
from concourse.bass_utils import run_bass_kernel_spmd

B, S, N, E = 16, 64, 2048, 256
NC = 8            # cores
ISL = N // NC     # 256 i per core
JT = N // 128     # 16 j-tiles
SA = S + 1        # 65 augmented contraction dim
G3 = 3 * ISL      # 768 gate-concat output per core
FP32 = mybir.dt.float32
AF = mybir.ActivationFunctionType
NEG = np.float32(-1e30)


# ------------------------------------------------------------------ host prep
def _host_prep(x, adj, Wq, bq, Wk, bk, Whr, bhr, Whz, bhz, Whn, bhn, Wo, bo, cbf16=False, mbf16=False):
    f64 = np.float64
    x = np.asarray(x, np.float32)

    G = np.asarray(Wq, f64).T @ np.asarray(Wk, f64)
    u = np.asarray(Wq, f64).T @ np.asarray(bk, f64)
    v = np.asarray(Wk, f64).T @ np.asarray(bq, f64)
    c = np.asarray(bq, f64) @ np.asarray(bk, f64)
    # out[j,i] = s[i,j] = xh_j^T M xh_i, M = [[G^T, v],[u^T, c]] (u pairs x_i, v pairs x_j)
    M = np.block([[G.T, v[:, None]], [u[None, :], np.array([[c]])]]).astype(np.float32)
    MT = np.ascontiguousarray(M.T)  # lhsT for H = M @ Xh_slice

    ones_row = np.ones((B, 1, N), np.float32)
    Xh = np.ascontiguousarray(np.concatenate([x, ones_row], axis=1))  # [B, 65, N]

    xT = np.transpose(x, (0, 2, 1))  # [B, N, S]
    xt_tiled = np.ascontiguousarray(
        xT.reshape(B, JT, 128, S).transpose(0, 2, 1, 3).reshape(B, 128, JT * S)
    )

    maskneg = np.where(np.asarray(adj) > 0, np.float32(0), NEG).astype(np.float32)
    I256 = np.eye(256, dtype=np.float32)
    I_tiled = np.ascontiguousarray(
        I256.reshape(2, 128, 256).transpose(1, 0, 2).reshape(128, 512)
    )

    Whs = [np.asarray(Whr, np.float32), np.asarray(Whz, np.float32), np.asarray(Whn, np.float32)]
    ball = np.concatenate([np.asarray(bhr), np.asarray(bhz), np.asarray(bhn)]).astype(np.float32)

    Wo_full = np.asarray(Wo, np.float32).reshape(N)
    Wo_tiled = np.ascontiguousarray(Wo_full.reshape(JT, 128).T)  # [128, 16]
    bo_val = np.asarray(bo, np.float32).reshape(1, 1)

    in_maps = []
    for cid in range(NC):
        isl = slice(cid * ISL, (cid + 1) * ISL)
        Wsl = np.concatenate([Wg.T[:, isl] for Wg in Whs], axis=1)  # [2048, 768]
        W_tiled = np.ascontiguousarray(
            Wsl.reshape(JT, 128, G3).transpose(1, 0, 2).reshape(128, JT * G3)
        )
        mask_tiled = np.ascontiguousarray(
            maskneg[isl, :].reshape(2, 128, N).transpose(1, 0, 2).reshape(128, 2 * N)
        )
        xhs = np.ascontiguousarray(Xh[:, :, isl])  # [B, 65, 256]
        b3 = np.concatenate(
            [ball[isl], ball[N + cid * ISL : N + (cid + 1) * ISL], np.zeros(ISL, np.float32)]
        )
        b3_rep = np.ascontiguousarray(np.broadcast_to(b3, (S, G3)))
        bn_rep = np.ascontiguousarray(
            np.broadcast_to(ball[2 * N + cid * ISL : 2 * N + (cid + 1) * ISL], (B, ISL))
        )
        wt, wot = W_tiled, Wo_tiled
        mt_, it_ = mask_tiled, I_tiled
        if cbf16:
            import ml_dtypes
            wt = W_tiled.astype(ml_dtypes.bfloat16)
            wot = Wo_tiled.astype(ml_dtypes.bfloat16)
        if mbf16:
            import ml_dtypes
            mt_ = mask_tiled.astype(ml_dtypes.bfloat16)
            it_ = I_tiled.astype(ml_dtypes.bfloat16)
        in_maps.append(
            dict(
                xh=Xh, xhs=xhs, xt=xt_tiled, mt=MT,
                mask=mt_, ident=it_, ident16=np.eye(B, dtype=np.float32), w=wt,
                b3=b3_rep, bn=bn_rep, wo=wot, bo=bo_val,
            )
        )
    return in_maps


# ------------------------------------------------------------------ v7 ------
HB = B // 2  # half-batch group size for the D AllReduce


def _host_prep_v7(x, adj, Wq, bq, Wk, bk, Whr, bhr, Whz, bhz, Whn, bhn, Wo, bo, dr=False):
    import ml_dtypes

    BF = ml_dtypes.bfloat16
    F8 = ml_dtypes.float8_e4m3
    DRS = 16.0   # fp8 weight scale
    DRT = 1024.0  # total GEMM scale under dr (weights x16, h x64)
    f64 = np.float64
    x = np.asarray(x, np.float32)

    G = np.asarray(Wq, f64).T @ np.asarray(Wk, f64)
    u = np.asarray(Wq, f64).T @ np.asarray(bk, f64)
    v = np.asarray(Wk, f64).T @ np.asarray(bq, f64)
    c = np.asarray(bq, f64) @ np.asarray(bk, f64)
    M = np.block([[G.T, v[:, None]], [u[None, :], np.array([[c]])]]).astype(np.float32)
    MT = np.ascontiguousarray(M.T)

    ones_row = np.ones((B, 1, N), np.float32)
    Xh = np.concatenate([x, ones_row], axis=1)  # [B, 65, N]
    Xh_bf = np.ascontiguousarray(Xh).astype(BF)

    xT = np.transpose(x, (0, 2, 1))  # [B, N, S]
    xt_bf = np.ascontiguousarray(
        xT.reshape(B, JT, 128, S).transpose(0, 2, 1, 3).reshape(B, 128, JT * S)
    ).astype(BF)

    mask01 = (np.asarray(adj) > 0).astype(np.float32)
    Whs = [np.asarray(Whr, np.float32), np.asarray(Whz, np.float32), np.asarray(Whn, np.float32)]
    ball = np.concatenate([np.asarray(bhr), np.asarray(bhz), np.asarray(bhn)]).astype(np.float32)

    Wo_full = np.asarray(Wo, np.float32).reshape(N)
    Wo_tiled = np.ascontiguousarray(Wo_full.reshape(JT, 128).T).astype(BF)  # [128, 16]
    bo_val = np.asarray(bo, np.float32).reshape(1, 1)

    in_maps = []
    for cid in range(NC):
        isl = slice(cid * ISL, (cid + 1) * ISL)
        Wsl = np.concatenate([Wg.T[:, isl] for Wg in Whs], axis=1)  # [2048, 768]
        W_tiled = np.ascontiguousarray(
            Wsl.reshape(JT, 128, G3).transpose(1, 0, 2).reshape(128, JT * G3)
        ).astype(BF)
        # mask01 for this i-slice, j-tiled: [j=2048, i=256] -> [128, 16*256]
        m_sl = np.ascontiguousarray(mask01[isl, :].T)  # [j, i_local]
        m_tiled = np.ascontiguousarray(
            m_sl.reshape(JT, 128, ISL).transpose(1, 0, 2).reshape(128, JT * ISL)
        ).astype(ml_dtypes.bfloat16)
        xhs = np.ascontiguousarray(Xh[:, :, isl])  # [B, 65, 256] fp32
        b2 = np.concatenate([ball[isl], ball[N + cid * ISL : N + (cid + 1) * ISL]])
        if dr:
            b2 = b2 * DRT
        b2_rep = np.ascontiguousarray(np.broadcast_to(b2, (S, 2 * ISL)))
        bnv = ball[2 * N + cid * ISL : 2 * N + (cid + 1) * ISL]
        if dr:
            bnv = bnv * DRT
        bn_row = np.ascontiguousarray(np.broadcast_to(bnv, (B, ISL)))
        m = dict(
            xh=Xh_bf, xhs=xhs, xt=xt_bf, mt=MT, mask=m_tiled, w=W_tiled,
            b2=b2_rep, bn=bn_row, ones16=np.ones((1, B), np.float32),
            ident16=np.eye(B, dtype=np.float32), ident16b=np.eye(B, dtype=BF), wo=Wo_tiled, bo=bo_val,
        )
        if dr:
            # fp8 DoubleRow packing: pair element j(p, e) = c*256 + 128*e + p
            NCH = N // 256
            W8 = np.zeros((128, NCH, 2, G3), np.float32)
            for c in range(NCH):
                for e in range(2):
                    W8[:, c, e, :] = DRS * Wsl[c * 256 + 128 * e : c * 256 + 128 * e + 128, :]
            m["w8"] = np.ascontiguousarray(W8.reshape(128, NCH * 2 * G3)).astype(F8)
            Wo8 = np.zeros((128, NCH, 2), np.float32)
            for c in range(NCH):
                for e in range(2):
                    Wo8[:, c, e] = DRS * Wo_full[c * 256 + 128 * e : c * 256 + 128 * e + 128]
            m["wo8"] = np.ascontiguousarray(Wo8.reshape(128, NCH * 2)).astype(F8)
            m["id16f8"] = np.eye(B, dtype=F8)
        in_maps.append(m)
    return in_maps


def _kernel_body_v7(tc, d, dr=False):
    nc = tc.nc
    RG = [list(range(NC))]
    BF16 = mybir.dt.bfloat16
    F8 = mybir.dt.float8e4
    DR = mybir.MatmulPerfMode.DoubleRow
    NCH = N // 256
    MM = nc.tensor.matmul

    with ExitStack() as ctx:
        const_pool = ctx.enter_context(tc.tile_pool(name="const", bufs=1))
        dram = ctx.enter_context(tc.tile_pool(name="dramscratch", bufs=1, space="DRAM"))

        # constants needed in phase C (loaded early on the gpsimd queue so the
        # sync queue is free for per-batch input streaming)
        if dr:
            w_sb = const_pool.tile([128, NCH * G3 * 2], F8)
            nc.gpsimd.dma_start(w_sb[:], d["w8"])
            wo_sb = const_pool.tile([128, NCH * 2], F8)
            nc.gpsimd.dma_start(wo_sb[:], d["wo8"])
            id16f8_sb = const_pool.tile([B, B], F8)
            nc.gpsimd.dma_start(id16f8_sb[:], d["id16f8"])
        else:
            w_sb = const_pool.tile([128, JT * G3], BF16)
            nc.gpsimd.dma_start(w_sb[:], d["w"])
            wo_sb = const_pool.tile([128, JT], BF16)
            nc.gpsimd.dma_start(wo_sb[:], d["wo"])
        bo_sb = const_pool.tile([1, 1], FP32)
        nc.gpsimd.dma_start(bo_sb[:], d["bo"])
        bn_sb = const_pool.tile([B, ISL], FP32)
        nc.gpsimd.dma_start(bn_sb[:], d["bn"])
        ones16_sb = const_pool.tile([1, B], FP32)
        nc.gpsimd.dma_start(ones16_sb[:], d["ones16"])
        id16_sb = const_pool.tile([B, B], FP32)
        nc.gpsimd.dma_start(id16_sb[:], d["ident16"])
        id16b_sb = const_pool.tile([B, B], BF16)
        nc.gpsimd.dma_start(id16b_sb[:], d["ident16b"])
        # phase A/B constants
        mask_sb = const_pool.tile([128, JT * ISL], BF16)
        nc.sync.dma_start(mask_sb[:], d["mask"])
        mt_sb = const_pool.tile([SA, SA], FP32)
        nc.sync.dma_start(mt_sb[:], d["mt"])
        b2_sb = const_pool.tile([S, 2 * ISL], FP32)
        nc.sync.dma_start(b2_sb[:], d["b2"])

        agg3_dram = dram.tile([B, S, G3], FP32)

        # ========================= phase A/B =========================
        with ExitStack() as actx:
            xh_pool = actx.enter_context(tc.tile_pool(name="xhp", bufs=2))
            small_pool = actx.enter_context(tc.tile_pool(name="smallp", bufs=2))
            e_pool = actx.enter_context(tc.tile_pool(name="ep", bufs=8))
            s_psum = actx.enter_context(tc.tile_pool(name="spsum", bufs=3, space="PSUM"))
            h_psum = actx.enter_context(tc.tile_pool(name="hpsum", bufs=2, space="PSUM"))
            g_psum = actx.enter_context(tc.tile_pool(name="gpsum", bufs=2, space="PSUM"))
            ar_dram = actx.enter_context(tc.tile_pool(name="ardram", bufs=2, space="DRAM"))

            QB = 4  # batches per pipeline group
            NQ = B // QB

            def scores_group(q):
                dall_sb = small_pool.tile([128, QB * JT], FP32, tag="dall")
                for bl in range(QB):
                    b = q * QB + bl
                    xh_sb = xh_pool.tile([SA, N], BF16, tag="xh")
                    nc.sync.dma_start(xh_sb[:], d["xh"][b])
                    xhs_sb = small_pool.tile([SA, ISL], FP32, tag="xhs")
                    nc.sync.dma_start(xhs_sb[:], d["xhs"][b])

                    h_ps = h_psum.tile([SA, ISL], FP32, tag="hps")
                    MM(h_ps[:], mt_sb[:], xhs_sb[:], start=True, stop=True)
                    h_sb = small_pool.tile([SA, ISL], BF16, tag="hsb")
                    nc.scalar.copy(h_sb[:], h_ps[:])

                    e_sb = e_pool.tile([128, JT * ISL], BF16, tag="esb")
                    e_tiles[b] = e_sb
                    for jp in range(JT // 2):
                        s_ps = s_psum.tile([128, 2 * ISL], FP32, tag="sps")
                        for k in range(2):
                            jt = 2 * jp + k
                            MM(
                                s_ps[:, k * ISL : (k + 1) * ISL],
                                xh_sb[:, jt * 128 : (jt + 1) * 128],
                                h_sb[:], start=True, stop=True,
                            )
                        eraw = small_pool.tile([128, 2 * ISL], BF16, tag="eraw")
                        nc.scalar.activation(eraw[:], s_ps[:], AF.Exp)
                        nc.vector.tensor_mul(
                            e_sb[:, jp * 2 * ISL : (jp + 1) * 2 * ISL],
                            eraw[:],
                            mask_sb[:, jp * 2 * ISL : (jp + 1) * 2 * ISL],
                        )
                    nc.vector.tensor_reduce(
                        dall_sb[:, bl * JT : (bl + 1) * JT],
                        e_sb[:].rearrange("p (j i) -> p j i", i=ISL),
                        axis=mybir.AxisListType.X, op=mybir.AluOpType.add,
                    )
                ar_in = ar_dram.tile([128, QB * JT], FP32, tag="arin")
                nc.sync.dma_start(ar_in[:], dall_sb[:])
                ar_out = ar_dram.tile([128, QB * JT], FP32, tag="arout")
                nc.gpsimd.collective_compute(
                    "AllReduce", mybir.AluOpType.add, replica_groups=RG,
                    ins=[ar_in.opt()], outs=[ar_out.opt()],
                )
                return ar_out

            def finish_group(q, ar_out):
                df_sb = small_pool.tile([128, QB * JT], FP32, tag="dfsb")
                nc.sync.dma_start(df_sb[:], ar_out[:])
                dinv_sb = small_pool.tile([128, QB * JT], FP32, tag="dinv")
                nc.vector.reciprocal(dinv_sb[:], df_sb[:])
                if dr:
                    dinv16 = small_pool.tile([128, QB * JT], FP32, tag="dinv16")
                    nc.scalar.mul(dinv16[:], dinv_sb[:], 1024.0)
                    return dinv16
                return dinv_sb

            def agg_group(q, dinv_sb):
                for bl in range(QB):
                    b = q * QB + bl
                    xt_sb = small_pool.tile([128, JT * S], BF16, tag="xt")
                    nc.sync.dma_start(xt_sb[:], d["xt"][b])
                    e_sb = e_tiles[b]
                    xd_sb = small_pool.tile([128, JT * S], BF16, tag="xdsb")
                    agg_ps = g_psum.tile([S, ISL], FP32, tag="aggps")
                    nc.vector.tensor_mul(
                        xd_sb[:].rearrange("p (j t) -> p j t", t=S),
                        xt_sb[:].rearrange("p (j t) -> p j t", t=S),
                        dinv_sb[:, bl * JT : (bl + 1) * JT, None].to_broadcast(
                            [128, JT, S]
                        ),
                    )
                    for jt in range(JT):
                        MM(
                            agg_ps[:], xd_sb[:, jt * S : (jt + 1) * S],
                            e_sb[:, jt * ISL : (jt + 1) * ISL],
                            start=(jt == 0), stop=(jt == JT - 1),
                        )
                    agg_sb = small_pool.tile([S, G3], FP32, tag="aggsb")
                    nc.vector.tensor_add(agg_sb[:, 0:ISL], agg_ps[:], b2_sb[:, 0:ISL])
                    nc.vector.tensor_add(
                        agg_sb[:, ISL : 2 * ISL], agg_ps[:], b2_sb[:, ISL : 2 * ISL]
                    )
                    nc.scalar.copy(agg_sb[:, 2 * ISL : G3], agg_ps[:])
                    nc.sync.dma_start(agg3_dram[b], agg_sb[:])

            e_tiles = {}
            prev = None
            for q in range(NQ):
                ar_out = scores_group(q)
                if prev is not None:
                    agg_group(prev[0], prev[1])
                dinv_sb = finish_group(q, ar_out)
                prev = (q, dinv_sb)
            agg_group(prev[0], prev[1])

        # ========================= phase C =========================
        # ht layout: [128, c*B ..] with global j-chunk c = 2*rank + cc.
        # AG payload per rank: [128, 2, B] p-major (64B contiguous per row).
        with ExitStack() as cctx:
            ht_pool = cctx.enter_context(tc.tile_pool(name="htp", bufs=2))
            gate_pool = cctx.enter_context(tc.tile_pool(name="gatep", bufs=2))
            aggt_pool = cctx.enter_context(tc.tile_pool(name="aggtp", bufs=3))
            seg_psum = cctx.enter_context(tc.tile_pool(name="segpsum", bufs=2, space="PSUM"))
            t_psum = cctx.enter_context(tc.tile_pool(name="tpsum", bufs=1, space="PSUM"))
            ag_dram = cctx.enter_context(tc.tile_pool(name="agdram", bufs=2, space="DRAM"))

            if dr:
                ht_sb = ht_pool.tile([128, NCH * 2 * B], F8, tag="ht")
                nc.vector.memset(ht_sb[:], 0.0)
                h_sb = gate_pool.tile([B, ISL], BF16, tag="hsl")
                nc.vector.memset(h_sb[:], 0.0)

                aggt_sb = aggt_pool.tile([B, G3], FP32, tag="aggt")
                nc.scalar.dma_start(aggt_sb[:], agg3_dram[:, 0, :])

                wv = w_sb[:].rearrange("p (c e n) -> p c e n", c=NCH, e=2)
                for t in range(S):
                    preA = seg_psum.tile([B, 2 * ISL], FP32, tag="preA")
                    preB = seg_psum.tile([B, ISL], FP32, tag="preB")
                    htv = ht_sb[:].rearrange("p (c e b) -> p c e b", c=NCH, e=2)
                    for c in range(NCH):
                        MM(
                            preA[:], htv[:, c],
                            wv[:, c, :, 0 : 2 * ISL],
                            start=(c == 0), stop=(c == NCH - 1), perf_mode=DR,
                        )
                    for c in range(NCH):
                        MM(
                            preB[:], htv[:, c],
                            wv[:, c, :, 2 * ISL : G3],
                            start=(c == 0), stop=(c == NCH - 1), perf_mode=DR,
                        )
                    if t + 1 < S:
                        aggt_next = aggt_pool.tile([B, G3], FP32, tag="aggt")
                        nc.scalar.dma_start(aggt_next[:], agg3_dram[:, t + 1, :])

                    rzin = gate_pool.tile([B, 2 * ISL], FP32, tag="rzin")
                    nc.vector.tensor_add(rzin[:], preA[:], aggt_sb[:, 0 : 2 * ISL])
                    rz = gate_pool.tile([B, 2 * ISL], FP32, tag="rz")
                    nc.scalar.activation(rz[:], rzin[:], AF.Sigmoid, scale=1.0 / 1024)
                    nt1 = gate_pool.tile([B, ISL], FP32, tag="nt1")
                    nc.vector.tensor_add(nt1[:], preB[:], bn_sb[:])
                    nt2 = gate_pool.tile([B, ISL], FP32, tag="nt2")
                    nc.vector.tensor_mul(nt2[:], nt1[:], rz[:, 0:ISL])
                    nin = gate_pool.tile([B, ISL], FP32, tag="nin")
                    nc.vector.tensor_add(nin[:], nt2[:], aggt_sb[:, 2 * ISL : G3])
                    ng = gate_pool.tile([B, ISL], FP32, tag="ng")
                    nc.scalar.activation(ng[:], nin[:], AF.Tanh, scale=1.0 / 1024)
                    hmn = gate_pool.tile([B, ISL], FP32, tag="hmn")
                    nc.vector.tensor_sub(hmn[:], h_sb[:], ng[:])
                    zh = gate_pool.tile([B, ISL], FP32, tag="zh")
                    nc.vector.tensor_mul(zh[:], rz[:, ISL : 2 * ISL], hmn[:])
                    h_new = gate_pool.tile([B, ISL], BF16, tag="hsl")
                    nc.vector.tensor_add(h_new[:], zh[:], ng[:])
                    h_sb = h_new
                    aggt_sb = aggt_next if t + 1 < S else aggt_sb

                    # transpose halves (bf16) -> [p, (e b)]; fp8 cast in copy
                    tp_ps = t_psum.tile([128, 2 * B], BF16, tag="tpps")
                    for e in range(2):
                        nc.tensor.transpose(
                            tp_ps[:, e * B : (e + 1) * B],
                            h_new[:, e * 128 : (e + 1) * 128], id16b_sb[:],
                        )
                    tp_sb = gate_pool.tile([128, 2 * B], F8, tag="tpsb")
                    nc.scalar.mul(tp_sb[:], tp_ps[:], 64.0)
                    ag_in = ag_dram.tile([128, 2 * B], F8, tag="agin")
                    nc.gpsimd.dma_start(ag_in[:], tp_sb[:])
                    ag_out = ag_dram.tile([NC * 128, 2 * B], F8, tag="agout", addr_space="Shared")
                    nc.gpsimd.collective_compute(
                        "AllGather", mybir.AluOpType.bypass, replica_groups=RG,
                        ins=[ag_in.opt()], outs=[ag_out.opt()],
                    )
                    ht_sb = ht_pool.tile([128, NCH * 2 * B], F8, tag="ht")
                    ag_v = ag_out[:].rearrange("(c p) x -> p c x", p=128)
                    ht_v = ht_sb[:].rearrange("p (c x) -> p c x", c=NCH)
                    nc.sync.dma_start(ht_v[:, 0:1, :], ag_v[:, 0:1, :])
                    nc.scalar.dma_start(ht_v[:, 1:2, :], ag_v[:, 1:2, :])
                    nc.gpsimd.dma_start(ht_v[:, 2:4, :], ag_v[:, 2:4, :])
                    nc.sync.dma_start(ht_v[:, 4:6, :], ag_v[:, 4:6, :])
                    nc.scalar.dma_start(ht_v[:, 6:8, :], ag_v[:, 6:8, :])

                out_ps = t_psum.tile([1, B], FP32, tag="outps")
                for ce in range(2 * NCH):
                    MM(
                        out_ps[:], wo_sb[:, ce : ce + 1],
                        ht_sb[:, ce * B : (ce + 1) * B],
                        start=(ce == 0), stop=(ce == 2 * NCH - 1),
                    )
                out_f = gate_pool.tile([1, B], FP32, tag="outf")
                nc.scalar.mul(out_f[:], out_ps[:], 1.0 / 1024)
                out_sb = gate_pool.tile([1, B], FP32, tag="outsb")
                nc.vector.tensor_scalar_add(out_sb[:], out_f[:], bo_sb[0:1, 0:1])
                nc.sync.dma_start(d["out"], out_sb[:])
                return

            ht_sb = ht_pool.tile([128, JT * B], BF16, tag="ht")
            nc.vector.memset(ht_sb[:], 0.0)
            h_sb = gate_pool.tile([B, ISL], BF16, tag="hsl")
            nc.vector.memset(h_sb[:], 0.0)

            aggt_sb = aggt_pool.tile([B, G3], FP32, tag="aggt")
            nc.scalar.dma_start(aggt_sb[:], agg3_dram[:, 0, :])

            KG = JT // 2  # k-chunks per column-tile group
            for t in range(S):
                # ---- gate GEMM: seg A (r|z cols 0:512), then seg B (n cols
                # 512:768); 2-way PE column tiling (groups stream concurrently)
                preA = seg_psum.tile([48, 2 * ISL], FP32, tag="preA")
                preB = seg_psum.tile([48, ISL], FP32, tag="preB")
                for k in range(KG):
                    for g in range(2):
                        jc = g * KG + k
                        MM(
                            preA[32 * g : 32 * g + B, :],
                            ht_sb[:, jc * B : (jc + 1) * B],
                            w_sb[:, jc * G3 : jc * G3 + 2 * ISL],
                            start=(k == 0), stop=(k == KG - 1),
                            tile_position=(0, 32 * g),
                        )
                for k in range(KG):
                    for g in range(2):
                        jc = g * KG + k
                        MM(
                            preB[32 * g : 32 * g + B, :],
                            ht_sb[:, jc * B : (jc + 1) * B],
                            w_sb[:, jc * G3 + 2 * ISL : (jc + 1) * G3],
                            start=(k == 0), stop=(k == KG - 1),
                            tile_position=(0, 32 * g),
                        )

                # prefetch next aggt (scalar queue)
                if t + 1 < S:
                    aggt_next = aggt_pool.tile([B, G3], FP32, tag="aggt")
                    nc.scalar.dma_start(aggt_next[:], agg3_dram[:, t + 1, :])

                # ---- gates: fp32 where a PSUM operand sets the DVE rate
                # anyway; bf16 on the pure-SBUF chain ops
                rzt = gate_pool.tile([B, 2 * ISL], FP32, tag="rzt")
                nc.vector.tensor_add(rzt[:], preA[32 : 32 + B, :], aggt_sb[:, 0 : 2 * ISL])
                rzin = gate_pool.tile([B, 2 * ISL], FP32, tag="rzin")
                nc.vector.tensor_add(rzin[:], preA[0:B, :], rzt[:])
                rz = gate_pool.tile([B, 2 * ISL], BF16, tag="rz")
                nc.scalar.activation(rz[:], rzin[:], AF.Sigmoid)
                zc = gate_pool.tile([B, ISL], BF16, tag="zc")
                nc.scalar.activation(
                    zc[:], rz[:, ISL : 2 * ISL], AF.Copy, bias=1.0, scale=-1.0
                )

                nt1a = gate_pool.tile([B, ISL], FP32, tag="nt1a")
                nc.vector.tensor_add(nt1a[:], preB[32 : 32 + B, :], bn_sb[:])
                nt1 = gate_pool.tile([B, ISL], FP32, tag="nt1")
                nc.vector.tensor_add(nt1[:], preB[0:B, :], nt1a[:])
                nt2 = gate_pool.tile([B, ISL], BF16, tag="nt2")
                nc.vector.tensor_mul(nt2[:], nt1[:], rz[:, 0:ISL])
                nin = gate_pool.tile([B, ISL], BF16, tag="nin")
                nc.vector.tensor_add(nin[:], nt2[:], aggt_sb[:, 2 * ISL : G3])
                zh2 = gate_pool.tile([B, ISL], BF16, tag="zh2")
                nc.vector.tensor_mul(zh2[:], rz[:, ISL : 2 * ISL], h_sb[:])
                ng = gate_pool.tile([B, ISL], BF16, tag="ng")
                nc.scalar.activation(ng[:], nin[:], AF.Tanh)
                t1 = gate_pool.tile([B, ISL], BF16, tag="t1")
                nc.vector.tensor_mul(t1[:], ng[:], zc[:])
                h_new = gate_pool.tile([B, ISL], BF16, tag="hsl")
                nc.vector.tensor_add(h_new[:], t1[:], zh2[:])
                h_sb = h_new
                aggt_sb = aggt_next if t + 1 < S else aggt_sb

                # ---- transpose h slice -> [128, 2*B], AllGather, reload ----
                tp_ps = t_psum.tile([128, 2 * B], BF16, tag="tpps")
                for cc in range(2):
                    nc.tensor.transpose(
                        tp_ps[:, cc * B : (cc + 1) * B],
                        h_new[:, cc * 128 : (cc + 1) * 128], id16b_sb[:],
                    )
                tp_sb = gate_pool.tile([128, 2 * B], BF16, tag="tpsb")
                nc.scalar.copy(tp_sb[:], tp_ps[:])
                ag_in = ag_dram.tile([2 * 128, B], BF16, tag="agin")
                ag_in_v = ag_in[:].rearrange("(p c) b -> p (c b)", c=2)
                nc.gpsimd.dma_start(ag_in_v[0:64, :], tp_sb[0:64, :])
                nc.sync.dma_start(ag_in_v[64:128, :], tp_sb[64:128, :])
                ag_out = ag_dram.tile([N, B], BF16, tag="agout", addr_space="Shared")
                nc.gpsimd.collective_compute(
                    "AllGather", mybir.AluOpType.bypass, replica_groups=RG,
                    ins=[ag_in.opt()], outs=[ag_out.opt()],
                )
                ht_sb = ht_pool.tile([128, JT * B], BF16, tag="ht")
                # ag_out rows: r*256 + p*2 + cc ; SBUF chunk c = 2r + cc
                ag_v = ag_out[:].rearrange("(r p c) b -> p r (c b)", p=128, c=2)
                ht_v = ht_sb[:].rearrange("p (r cb) -> p r cb", r=NC)
                nc.sync.dma_start(ht_v[:, 0:8:4, :], ag_v[:, 0:8:4, :])
                nc.scalar.dma_start(ht_v[:, 1:8:4, :], ag_v[:, 1:8:4, :])
                nc.gpsimd.dma_start(ht_v[:, 2:8:4, :], ag_v[:, 2:8:4, :])
                nc.sync.dma_start(ht_v[:, 3:8:4, :], ag_v[:, 3:8:4, :])

            # output head
            out_ps = t_psum.tile([1, B], FP32, tag="outps")
            for jc in range(JT):
                MM(
                    out_ps[:], wo_sb[:, jc : jc + 1], ht_sb[:, jc * B : (jc + 1) * B],
                    start=(jc == 0), stop=(jc == JT - 1),
                )
            out_sb = gate_pool.tile([1, B], FP32, tag="outsb")
            nc.vector.tensor_scalar_add(out_sb[:], out_ps[:], bo_sb[0:1, 0:1])
            nc.sync.dma_start(d["out"], out_sb[:])


def _build_v7(dr=False):
    nc = bacc.Bacc("TRN2", target_bir_lowering=False, debug=False, num_devices=NC)
    BF16 = mybir.dt.bfloat16
    F8 = mybir.dt.float8e4
    d = dict(
        xh=nc.dram_tensor("xh", [B, SA, N], BF16, kind="ExternalInput").ap(),
        xhs=nc.dram_tensor("xhs", [B, SA, ISL], FP32, kind="ExternalInput").ap(),
        xt=nc.dram_tensor("xt", [B, 128, JT * S], BF16, kind="ExternalInput").ap(),
        mt=nc.dram_tensor("mt", [SA, SA], FP32, kind="ExternalInput").ap(),
        mask=nc.dram_tensor("mask", [128, JT * ISL], BF16, kind="ExternalInput").ap(),
        w=nc.dram_tensor("w", [128, JT * G3], BF16, kind="ExternalInput").ap(),
        b2=nc.dram_tensor("b2", [S, 2 * ISL], FP32, kind="ExternalInput").ap(),
        bn=nc.dram_tensor("bn", [B, ISL], FP32, kind="ExternalInput").ap(),
        ones16=nc.dram_tensor("ones16", [1, B], FP32, kind="ExternalInput").ap(),
        ident16=nc.dram_tensor("ident16", [B, B], FP32, kind="ExternalInput").ap(),
        ident16b=nc.dram_tensor("ident16b", [B, B], BF16, kind="ExternalInput").ap(),
        wo=nc.dram_tensor("wo", [128, JT], BF16, kind="ExternalInput").ap(),
        bo=nc.dram_tensor("bo", [1, 1], FP32, kind="ExternalInput").ap(),
        out=nc.dram_tensor("out", [1, B], FP32, kind="ExternalOutput").ap(),
    )
    if dr:
        d["w8"] = nc.dram_tensor("w8", [128, (N // 256) * G3 * 2], F8, kind="ExternalInput").ap()
        d["wo8"] = nc.dram_tensor("wo8", [128, (N // 256) * 2], F8, kind="ExternalInput").ap()
        d["id16f8"] = nc.dram_tensor("id16f8", [B, B], F8, kind="ExternalInput").ap()
    with tile.TileContext(nc) as tc:
        _kernel_body_v7(tc, d, dr=dr)
    nc.compile()
    return nc


# ------------------------------------------------------------------ kernel IR
def _kernel_body(tc, d, variant="abc"):
    nc = tc.nc
    RG = [list(range(NC))]
    # variant features
    f_ab_only = variant == "ab"
    f_ag = variant not in ("noag", "cgemm", "cgates", "o5na")  # real collective AG
    f_warm = variant in ("o5warm",)  # dummy PE work during AG gap (HAM warmth)
    f_gemm = variant != "cgates"
    f_gates = variant not in ("cgemm",)
    f_opt = variant == "opt1"  # (legacy serial col-tiling bundle)
    f_ct = variant in ("opt6",)   # interleaved 2-way column-tiled GEMM
    f_ash = variant in ("opt6",)  # AllGather output in Shared scratchpad
    f_sdma = variant in ("opt6",) # split hT reload DMA
    f_mc = variant in ("opt6",)   # merged transpose copy
    f_r = variant in ("opt2",)  # float32r matmul operands (4x PE rate)
    OPT5F = ("opt3", "opt4", "opt5", "opt6", "o5na", "o5warm")
    f_cbf = variant in OPT5F  # phase-C GEMM + h in bf16
    f_mbf = variant in OPT5F[1:]  # mask/identity matmuls in bf16 (exact)
    f_abf = variant in ("opt5", "opt6", "o5na", "o5warm")  # AGG matmul operands in bf16
    f_ar = variant in ("opt4",)   # AGG matmul operands in float32r (HW-broken)
    f_sr = variant in ("opt4",)   # scores matmul operands in float32r (HW-broken)
    BF16 = mybir.dt.bfloat16
    F32R = mybir.dt.float32r
    CDT = BF16 if f_cbf else FP32
    MDT = BF16 if f_mbf else FP32

    def MM(out, lhsT, rhs, **kw):
        if f_r:
            lhsT = lhsT.bitcast(mybir.dt.float32r)
            rhs = rhs.bitcast(mybir.dt.float32r)
        nc.tensor.matmul(out, lhsT, rhs, **kw)

    if variant == "nop":
        # timing calibration: minimal kernel (one tiny DMA in/out)
        with tc.tile_pool(name="nopp", bufs=1) as nopp:
            fo = nopp.tile([1, B], FP32)
            nc.sync.dma_start(fo[:], d["bo"][0:1, 0:1].to_broadcast([1, B]))
            nc.sync.dma_start(d["out"], fo[:])
        return

    with ExitStack() as ctx:
        const_pool = ctx.enter_context(tc.tile_pool(name="const", bufs=1))
        dram = ctx.enter_context(tc.tile_pool(name="dramscratch", bufs=1, space="DRAM"))

        mask_sb = const_pool.tile([128, 2 * N], MDT)
        nc.sync.dma_start(mask_sb[:], d["mask"])
        id_sb = const_pool.tile([128, 512], MDT)
        nc.sync.dma_start(id_sb[:], d["ident"])
        id16_sb = const_pool.tile([B, B], FP32)
        nc.sync.dma_start(id16_sb[:], d["ident16"])
        mt_sb = const_pool.tile([SA, SA], FP32)
        nc.sync.dma_start(mt_sb[:], d["mt"])
        b3_sb = const_pool.tile([S, G3], FP32)
        nc.sync.dma_start(b3_sb[:], d["b3"])
        bn_sb = const_pool.tile([B, ISL], FP32)
        nc.sync.dma_start(bn_sb[:], d["bn"])
        wo_sb = const_pool.tile([128, JT], CDT)
        nc.sync.dma_start(wo_sb[:], d["wo"])
        bo_sb = const_pool.tile([1, 1], FP32)
        nc.sync.dma_start(bo_sb[:], d["bo"])
        w_sb = const_pool.tile([128, JT * G3], CDT)
        nc.sync.dma_start(w_sb[:], d["w"])

        agg3_dram = dram.tile([B, S, G3], FP32)

        # ========================= phase A/B =========================
        with ExitStack() as actx:
            xh_pool = actx.enter_context(tc.tile_pool(name="xhp", bufs=2))
            small_pool = actx.enter_context(tc.tile_pool(name="smallp", bufs=2))
            e_pool = actx.enter_context(tc.tile_pool(name="ep", bufs=2))
            s_psum = actx.enter_context(tc.tile_pool(name="spsum", bufs=3, space="PSUM"))
            h_psum = actx.enter_context(tc.tile_pool(name="hpsum", bufs=2, space="PSUM"))
            g_psum = actx.enter_context(tc.tile_pool(name="gpsum", bufs=2, space="PSUM"))
            ar_dram = actx.enter_context(tc.tile_pool(name="ardram", bufs=2, space="DRAM"))

            for b in range(B):
                xh_sb = xh_pool.tile([SA, N], FP32, tag="xh")
                nc.sync.dma_start(xh_sb[:], d["xh"][b])
                xhs_sb = small_pool.tile([SA, ISL], FP32, tag="xhs")
                nc.sync.dma_start(xhs_sb[:], d["xhs"][b])
                xt_sb = small_pool.tile([128, JT * S], FP32, tag="xt")
                nc.sync.dma_start(xt_sb[:], d["xt"][b])

                # H = M @ Xh[:, islice]  -> [65, 256]
                h_ps = h_psum.tile([SA, ISL], FP32, tag="hps")
                MM(h_ps[:], mt_sb[:], xhs_sb[:], start=True, stop=True)
                h_sb = small_pool.tile([SA, ISL], F32R if f_sr else FP32, tag="hsb")
                nc.scalar.copy(h_sb[:], h_ps[:])
                if f_sr:
                    xh_r = xh_pool.tile([SA, N], F32R, tag="xhr")
                    nc.scalar.copy(xh_r[:], xh_sb[:])
                else:
                    xh_r = xh_sb

                # E tiles: e_sb[p, jt*256 + i] = exp(s[i, jt*128+p] + maskneg)
                e_sb = e_pool.tile(
                    [128, JT * ISL], BF16 if f_abf else (F32R if f_ar else FP32), tag="esb"
                )
                for jt in range(JT):
                    s_ps = s_psum.tile([128, ISL], FP32, tag="sps")
                    MM(
                        s_ps[:], mask_sb[:, jt * 128 : (jt + 1) * 128],
                        id_sb[:, 0:ISL], start=True, stop=False,
                    )
                    MM(
                        s_ps[:], mask_sb[:, N + jt * 128 : N + (jt + 1) * 128],
                        id_sb[:, ISL : 2 * ISL], start=False, stop=False,
                    )
                    nc.tensor.matmul(
                        s_ps[:], xh_r[:, jt * 128 : (jt + 1) * 128],
                        h_sb[:], start=False, stop=True,
                    )
                    nc.scalar.activation(
                        e_sb[:, jt * ISL : (jt + 1) * ISL], s_ps[:], AF.Exp
                    )

                # D partial = sum_i E  (one 3D reduce)
                d_sb = small_pool.tile([128, JT], FP32, tag="dsb")
                e_red = e_sb[:].bitcast(FP32) if f_ar else e_sb[:]
                nc.vector.tensor_reduce(
                    d_sb[:], e_red.rearrange("p (j i) -> p j i", i=ISL),
                    axis=mybir.AxisListType.X, op=mybir.AluOpType.add,
                )
                ar_in = ar_dram.tile([128, JT], FP32, tag="arin")
                nc.sync.dma_start(ar_in[:], d_sb[:])
                ar_out = ar_dram.tile([128, JT], FP32, tag="arout")
                nc.gpsimd.collective_compute(
                    "AllReduce", mybir.AluOpType.add, replica_groups=RG,
                    ins=[ar_in.opt()], outs=[ar_out.opt()],
                )
                df_sb = small_pool.tile([128, JT], FP32, tag="dfsb")
                nc.sync.dma_start(df_sb[:], ar_out[:])
                dinv_sb = small_pool.tile([128, JT], FP32, tag="dinv")
                nc.vector.reciprocal(dinv_sb[:], df_sb[:])

                # AGG[t, i] = sum_j (xT[j,t] * Dinv[j]) E[j, i]
                xd_sb = small_pool.tile(
                    [128, JT * S], BF16 if f_abf else (F32R if f_ar else FP32), tag="xdsb"
                )
                agg_ps = g_psum.tile([S, ISL], FP32, tag="aggps")
                for jt in range(JT):
                    nc.vector.tensor_scalar_mul(
                        xd_sb[:, jt * S : (jt + 1) * S],
                        xt_sb[:, jt * S : (jt + 1) * S],
                        dinv_sb[:, jt : jt + 1],
                    )
                    MM(
                        agg_ps[:], xd_sb[:, jt * S : (jt + 1) * S],
                        e_sb[:, jt * ISL : (jt + 1) * ISL],
                        start=(jt == 0), stop=(jt == JT - 1),
                    )

                # agg3 = [agg + bhr | agg + bhz | agg] -> DRAM[b]
                agg_sb = small_pool.tile([S, G3], FP32, tag="aggsb")
                nc.vector.tensor_add(agg_sb[:, 0:ISL], agg_ps[:], b3_sb[:, 0:ISL])
                nc.vector.tensor_add(
                    agg_sb[:, ISL : 2 * ISL], agg_ps[:], b3_sb[:, ISL : 2 * ISL]
                )
                nc.scalar.copy(agg_sb[:, 2 * ISL : G3], agg_ps[:])
                nc.sync.dma_start(agg3_dram[b], agg_sb[:])

        if f_ab_only:
            # timing variant: stop after phase A/B; emit a tiny output read
            with tc.tile_pool(name="fin", bufs=1) as fin:
                fo = fin.tile([1, B], FP32)
                nc.sync.dma_start(fo[:], agg3_dram[0, 0:1, 0:B])
                nc.sync.dma_start(d["out"], fo[:])
            return

        # ========================= phase C =========================
        with ExitStack() as cctx:
            ht_pool = cctx.enter_context(tc.tile_pool(name="htp", bufs=2))
            gate_pool = cctx.enter_context(tc.tile_pool(name="gatep", bufs=2))
            aggt_pool = cctx.enter_context(tc.tile_pool(name="aggtp", bufs=3))
            c_psum = cctx.enter_context(tc.tile_pool(name="cpsum", bufs=2, space="PSUM"))
            t_psum = cctx.enter_context(tc.tile_pool(name="tpsum", bufs=1, space="PSUM"))
            ag_dram = cctx.enter_context(tc.tile_pool(name="agdram", bufs=2, space="DRAM"))

            ht_sb = ht_pool.tile([128, JT * B], CDT, tag="ht")
            nc.vector.memset(ht_sb[:], 0.0)
            h_sb = gate_pool.tile([B, ISL], FP32, tag="hsl")
            nc.vector.memset(h_sb[:], 0.0)

            aggt_sb = aggt_pool.tile([B, G3], FP32, tag="aggt")
            nc.sync.dma_start(aggt_sb[:], agg3_dram[:, 0, :])

            if not f_gemm:
                pre_fix = c_psum.tile([B, G3], FP32, tag="prefix")
                nc.vector.memset(pre_fix[:], 0.0)

            for t in range(S):
                # ---- gate GEMM ----
                if f_gemm and f_ct:
                    # interleaved 2-way column tiling: adjacent MMs alternate
                    # PE column groups so they stream concurrently
                    pre_ps = c_psum.tile([48, G3], FP32, tag="preps")
                    for k in range(8):
                        for seg0, seg1 in ((0, 512), (512, G3)):
                            for grp in (0, 1):
                                jc = grp * 8 + k
                                rows = pre_ps[32 * grp : 32 * grp + B, :]
                                lhsT = ht_sb[:, jc * B : (jc + 1) * B]
                                MM(
                                    rows[:, seg0:seg1], lhsT,
                                    w_sb[:, jc * G3 + seg0 : jc * G3 + seg1],
                                    start=(k == 0), stop=(k == 7),
                                    tile_position=(0, 32 * grp),
                                )
                elif f_gemm and f_opt:
                    # 2-way PE column-tiling: j-chunks 0-7 -> col group 0
                    # (psum rows 0:16), chunks 8-15 -> col group 1 (rows 32:48)
                    pre_ps = c_psum.tile([48, G3], FP32, tag="preps")
                    for jc in range(JT):
                        grp = jc // 8
                        rows = pre_ps[32 * grp : 32 * grp + B, :]
                        lhsT = ht_sb[:, jc * B : (jc + 1) * B]
                        MM(
                            rows[:, 0:512], lhsT, w_sb[:, jc * G3 : jc * G3 + 512],
                            start=(jc % 8 == 0), stop=(jc % 8 == 7),
                            tile_position=(0, 32 * grp),
                        )
                        MM(
                            rows[:, 512:G3], lhsT, w_sb[:, jc * G3 + 512 : (jc + 1) * G3],
                            start=(jc % 8 == 0), stop=(jc % 8 == 7),
                            tile_position=(0, 32 * grp),
                        )
                elif f_gemm:
                    pre_ps = c_psum.tile([B, G3], FP32, tag="preps")
                    for jc in range(JT):
                        lhsT = ht_sb[:, jc * B : (jc + 1) * B]
                        MM(
                            pre_ps[:, 0:512], lhsT, w_sb[:, jc * G3 : jc * G3 + 512],
                            start=(jc == 0), stop=(jc == JT - 1),
                        )
                        MM(
                            pre_ps[:, 512:G3], lhsT, w_sb[:, jc * G3 + 512 : (jc + 1) * G3],
                            start=(jc == 0), stop=(jc == JT - 1),
                        )
                else:
                    pre_ps = pre_fix

                # prefetch next agg (off critical path, SWDGE queue)
                if t + 1 < S:
                    aggt_next = aggt_pool.tile([B, G3], FP32, tag="aggt")
                    nc.gpsimd.dma_start(aggt_next[:], agg3_dram[:, t + 1, :])

                # ---- gates ----
                if f_gates:
                    rzin = gate_pool.tile([B, 2 * ISL], FP32, tag="rzin")
                    if f_opt or f_ct:
                        # merge col-group partials inside the adds (one PSUM
                        # operand per DVE op)
                        rzt = gate_pool.tile([B, 2 * ISL], FP32, tag="rzt")
                        nc.vector.tensor_add(
                            rzt[:], pre_ps[32 : 32 + B, 0 : 2 * ISL], aggt_sb[:, 0 : 2 * ISL]
                        )
                        nc.vector.tensor_add(rzin[:], pre_ps[0:B, 0 : 2 * ISL], rzt[:])
                    else:
                        nc.vector.tensor_add(
                            rzin[:], pre_ps[0:B, 0 : 2 * ISL], aggt_sb[:, 0 : 2 * ISL]
                        )
                    rz = gate_pool.tile([B, 2 * ISL], FP32, tag="rz")
                    nc.scalar.activation(rz[:], rzin[:], AF.Sigmoid)
                    nt1 = gate_pool.tile([B, ISL], FP32, tag="nt1")
                    if f_opt or f_ct:
                        nt1a = gate_pool.tile([B, ISL], FP32, tag="nt1a")
                        nc.vector.tensor_add(
                            nt1a[:], pre_ps[32 : 32 + B, 2 * ISL : G3], bn_sb[:]
                        )
                        nc.vector.tensor_add(nt1[:], pre_ps[0:B, 2 * ISL : G3], nt1a[:])
                    else:
                        nc.vector.tensor_add(nt1[:], pre_ps[0:B, 2 * ISL : G3], bn_sb[:])
                    nt2 = gate_pool.tile([B, ISL], FP32, tag="nt2")
                    nc.vector.tensor_mul(nt2[:], nt1[:], rz[:, 0:ISL])
                    nin = gate_pool.tile([B, ISL], FP32, tag="nin")
                    nc.vector.tensor_add(nin[:], nt2[:], aggt_sb[:, 2 * ISL : G3])
                    ng = gate_pool.tile([B, ISL], FP32, tag="ng")
                    nc.scalar.activation(ng[:], nin[:], AF.Tanh)
                    hmn = gate_pool.tile([B, ISL], FP32, tag="hmn")
                    nc.vector.tensor_sub(hmn[:], h_sb[:], ng[:])
                    zh = gate_pool.tile([B, ISL], FP32, tag="zh")
                    nc.vector.tensor_mul(zh[:], rz[:, ISL : 2 * ISL], hmn[:])
                    h_new = gate_pool.tile([B, ISL], FP32, tag="hsl")
                    nc.vector.tensor_add(h_new[:], zh[:], ng[:])
                else:
                    # timing variant: single bounded op stands in for the gates
                    h_new = gate_pool.tile([B, ISL], FP32, tag="hsl")
                    nc.scalar.activation(h_new[:], pre_ps[0:B, 0:ISL], AF.Tanh)
                h_sb = h_new
                aggt_sb = aggt_next if t + 1 < S else aggt_sb

                # ---- transpose h slice -> [128, 16] x2, AllGather, reload hT ----
                tp_sb = gate_pool.tile([128, 2 * B], CDT, tag="tpsb")
                if f_opt or f_mc:
                    tp_ps = t_psum.tile([128, 2 * B], BF16, tag="tpps")
                    for cch in range(2):
                        nc.tensor.transpose(
                            tp_ps[:, cch * B : (cch + 1) * B],
                            h_new[:, cch * 128 : (cch + 1) * 128], id16_sb[:],
                        )
                    nc.scalar.mul(tp_sb[:], tp_ps[:], 64.0)
                else:
                    for cch in range(2):
                        tp_ps = t_psum.tile([128, B], FP32, tag="tpps")
                        nc.tensor.transpose(
                            tp_ps[:], h_new[:, cch * 128 : (cch + 1) * 128], id16_sb[:]
                        )
                        nc.scalar.copy(tp_sb[:, cch * B : (cch + 1) * B], tp_ps[:])
                if f_warm:
                    # keep the PE HAM clock warm through the AllGather gap:
                    # chained junk matmuls gated on tp_sb (i.e. after the
                    # transposes) accumulating into a scratch PSUM bank
                    warm_ps = t_psum.tile([B, 512], FP32, tag="warmps")
                    for wi in range(16):
                        nc.tensor.matmul(
                            warm_ps[:], tp_sb[:, 0:B],
                            w_sb[:, (wi % JT) * G3 : (wi % JT) * G3 + 512],
                            start=(wi == 0), stop=(wi == 15),
                        )
                ag_in = ag_dram.tile([2 * 128, B], CDT, tag="agin")
                nc.sync.dma_start(
                    ag_in[:].rearrange("(c p) b -> p c b", p=128),
                    tp_sb[:].rearrange("p (c b) -> p c b", c=2),
                )
                ag_out = ag_dram.tile(
                    [N, B], CDT, tag="agout",
                    addr_space=("Shared" if (f_opt or f_ash) else "Local"),
                )
                if f_ag:
                    nc.gpsimd.collective_compute(
                        "AllGather", mybir.AluOpType.bypass, replica_groups=RG,
                        ins=[ag_in.opt()], outs=[ag_out.opt()],
                    )
                else:
                    # timing variant: local DRAM->DRAM copy of per-rank size
                    nc.sync.dma_start(ag_out[0 : 2 * 128, :], ag_in[:])
                ht_sb = ht_pool.tile([128, JT * B], CDT, tag="ht")
                if f_opt or f_sdma:
                    # split reload so the first GEMM chunks can start earlier
                    for half in range(2):
                        nc.sync.dma_start(
                            ht_sb[:, half * 8 * B : (half + 1) * 8 * B].rearrange(
                                "p (c b) -> p c b", c=8
                            ),
                            ag_out[half * 1024 : (half + 1) * 1024, :].rearrange(
                                "(c p) b -> p c b", p=128
                            ),
                        )
                else:
                    nc.sync.dma_start(
                        ht_sb[:].rearrange("p (c b) -> p c b", c=JT),
                        ag_out[:].rearrange("(c p) b -> p c b", p=128),
                    )

            # output head: out[b] = sum_j h[b, j] Wo[j] + bo  (full h from last AG)
            out_ps = t_psum.tile([1, B], FP32, tag="outps")
            for jc in range(JT):
                nc.tensor.matmul(
                    out_ps[:], wo_sb[:, jc : jc + 1], ht_sb[:, jc * B : (jc + 1) * B],
                    start=(jc == 0), stop=(jc == JT - 1),
                )
            out_sb = gate_pool.tile([1, B], FP32, tag="outsb")
            nc.vector.tensor_scalar_add(out_sb[:], out_ps[:], bo_sb[0:1, 0:1])
            nc.sync.dma_start(d["out"], out_sb[:])


def _build(variant="abc"):
    nc = bacc.Bacc("TRN2", target_bir_lowering=False, debug=False, num_devices=NC)
    CDT = mybir.dt.bfloat16 if variant in ("opt3", "opt4", "opt5", "opt6", "o5na", "o5warm") else FP32
    MDT = mybir.dt.bfloat16 if variant in ("opt4", "opt5", "opt6", "o5na", "o5warm") else FP32
    d = dict(
        xh=nc.dram_tensor("xh", [B, SA, N], FP32, kind="ExternalInput").ap(),
        xhs=nc.dram_tensor("xhs", [B, SA, ISL], FP32, kind="ExternalInput").ap(),
        xt=nc.dram_tensor("xt", [B, 128, JT * S], FP32, kind="ExternalInput").ap(),
        mt=nc.dram_tensor("mt", [SA, SA], FP32, kind="ExternalInput").ap(),
        mask=nc.dram_tensor("mask", [128, 2 * N], MDT, kind="ExternalInput").ap(),
        ident=nc.dram_tensor("ident", [128, 512], MDT, kind="ExternalInput").ap(),
        ident16=nc.dram_tensor("ident16", [B, B], FP32, kind="ExternalInput").ap(),
        w=nc.dram_tensor("w", [128, JT * G3], CDT, kind="ExternalInput").ap(),
        b3=nc.dram_tensor("b3", [S, G3], FP32, kind="ExternalInput").ap(),
        bn=nc.dram_tensor("bn", [B, ISL], FP32, kind="ExternalInput").ap(),
        wo=nc.dram_tensor("wo", [128, JT], CDT, kind="ExternalInput").ap(),
        bo=nc.dram_tensor("bo", [1, 1], FP32, kind="ExternalInput").ap(),
        out=nc.dram_tensor("out", [1, B], FP32, kind="ExternalOutput").ap(),
    )
    with tile.TileContext(nc) as tc:
        _kernel_body(tc, d, variant=variant)
    nc.compile()  # bacc register allocation / DCE / fusion
    return nc


def prep_and_build(inputs, variant="v7"):
    if variant == "v7":
        return _host_prep_v7(**inputs), _build_v7()
    if variant == "v8":
        return _host_prep_v7(**inputs, dr=True), _build_v7(dr=True)
    cbf = variant in ("opt3", "opt4", "opt5", "opt6", "o5na", "o5warm")
    mbf = variant in ("opt4", "opt5", "opt6", "o5na", "o5warm")
    in_maps = _host_prep(**inputs, cbf16=cbf, mbf16=mbf)
    nc = _build(variant)
    return in_maps, nc


def run_with_results(inputs, trace=False, variant="abc", **kw):
    in_maps, nc = prep_and_build(inputs, variant)
    res = run_bass_kernel_spmd(
        nc, in_maps, core_ids=list(range(NC)), trace=trace, **kw
    )
    out = np.asarray(res.results[0]["out"], np.float32).reshape(B)
    return out, res


def kernel(**inputs) -> np.ndarray:
    out, _ = run_with_results(inputs, variant="v7")
    return out


if __name__ == "__main__":
    import reference

    inputs = {k: np.asarray(v) for k, v in reference.setup_inputs().items()}
    out = kernel(**inputs)
    print("kernel out:", out)



# revision 48
# speedup vs baseline: 1.0138x; 1.0138x over previous
"""Trainium2 Bass kernel for nn_AttGRU (B=16, S=64, N=2048, E=256) on 8 NeuronCores.

Default variant "v7" (legacy variants kept below for reference):
  - scores via the K=65 trick: sT[j,i] = xh_j^T M xh_i with M = 65x65 host-
    precomputed from Wq/Wk/bq/bk; scores matmuls in bf16.
  - phase A/B (attention + AGG precompute), i-sharded 8 ways, pipelined in
    batch-quarters: scores(q) overlaps AllReduce(q-1) and AGG(q-1), so the 4
    D-AllReduces are off the critical path. Mask applied as a bf16 0/1
    multiply after exp (no mask matmuls); D via one DVE 3D reduce per batch.
  - phase C (GRU, 64 sequential steps), i-sharded: per step one gate GEMM
    (bf16, 2-way PE column tiling, r|z segment then n segment so the rz gate
    math overlaps the n GEMM), gates on DVE/ACT (fp32 where a PSUM operand
    sets the rate, bf16 elsewhere), PE transposes of the h slice, and an 8KB
    bf16 AllGather with a p-major payload layout chosen so the SBUF->DRAM
    bounce and the reload are contiguous-per-partition DMAs split across the
    sync/scalar/gpsimd queues.
  - numerics: bf16 scores/E/AGG/h, fp32 PSUM accumulation everywhere;
    rel err vs fp32 reference ~8.5e-3 (tolerance 2e-2), deterministic.
  - phase C step details: h update computed as (1-z)*n + z*h with zc=1-z and
    z*h prepared during ACT/DVE slack so only two DVE ops follow tanh; reload
    as four stride-4 rank-pair DMAs so the first GEMM k-pair waits on a
    single semaphore; the 8KB SBUF->DRAM bounce split across two queues.
  - phase A/B AGG: the 16 per-jt x*Dinv scalar-muls are one DVE multiply
    with a stride-0 broadcast operand (dinv[:, jt, None].to_broadcast), since
    DVE tensor ops accept broadcast SBUF APs directly.
  - measured (NTFF): ~1.52 ms total = ~0.24 ms phase A/B + ~1.28 ms
    phase C (64 steps x ~20 us; step = reload 2.9 + GEMM 4.2 + gates 3.6 +
    transpose/copy 0.6 + bounce+trigger 3.2 + AllGather ~5.2).
    Baseline "abc" variant: 3.30 ms by the same measurement.
    Remaining A/B cap is DVE element throughput (masks 3.3us + D-reduce
    4.4us per batch); remaining C cap is collective latency (~11.3us/step).
  - negative results (all measured): 4-way PE column tiling (quadrant 3 is
    HW-broken), single-stream GEMM (2-way col tiling IS concurrent), any
    tensor ops on the Pool engine (phase C gates AND A/B masks/xd both
    regressed ~100-150us), HAM warm-up matmul blocks, fp8 h transport
    (3.1e-2 err: HW fp8 conversion loses more than ml_dtypes RNE predicts --
    also why variant "v8" fp8 DoubleRow, ~1.54ms, fails at 4e-2; both
    unused), s_psum bufs 3->4 (noise). A dependency-spaced PE ping chain
    covering the entire AllGather gap proved the GEMM's ~0.83ns/col per
    stream is the fixed column-tiled feed rate, NOT HAM throttling: the
    gate GEMM is at its rate ceiling for this shape.
"""

import sys

for _p in ("/opt/trn_rl_repo", "/root/.axon_site/_ro/trn_rl_repo"):
    if _p not in sys.path:
        sys.path.append(_p)

import numpy as np
from contextlib import ExitStack

import concourse.bacc as bacc
import concourse.tile as tile
import concourse.mybir as mybir
from concourse.bass_utils import run_bass_kernel_spmd

B, S, N, E = 16, 64, 2048, 256
NC = 8            # cores
ISL = N // NC     # 256 i per core
JT = N // 128     # 16 j-tiles
SA = S + 1        # 65 augmented contraction dim
G3 = 3 * ISL      # 768 gate-concat output per core
FP32 = mybir.dt.float32
AF = mybir.ActivationFunctionType
NEG = np.float32(-1e30)


# ------------------------------------------------------------------ host prep
def _host_prep(x, adj, Wq, bq, Wk, bk, Whr, bhr, Whz, bhz, Whn, bhn, Wo, bo, cbf16=False, mbf16=False):
    f64 = np.float64
    x = np.asarray(x, np.float32)

    G = np.asarray(Wq, f64).T @ np.asarray(Wk, f64)
    u = np.asarray(Wq, f64).T @ np.asarray(bk, f64)
    v = np.asarray(Wk, f64).T @ np.asarray(bq, f64)
    c = np.asarray(bq, f64) @ np.asarray(bk, f64)
    # out[j,i] = s[i,j] = xh_j^T M xh_i, M = [[G^T, v],[u^T, c]] (u pairs x_i, v pairs x_j)
    M = np.block([[G.T, v[:, None]], [u[None, :], np.array([[c]])]]).astype(np.float32)
    MT = np.ascontiguousarray(M.T)  # lhsT for H = M @ Xh_slice

    ones_row = np.ones((B, 1, N), np.float32)
    Xh = np.ascontiguousarray(np.concatenate([x, ones_row], axis=1))  # [B, 65, N]

    xT = np.transpose(x, (0, 2, 1))  # [B, N, S]
    xt_tiled = np.ascontiguousarray(
        xT.reshape(B, JT, 128, S).transpose(0, 2, 1, 3).reshape(B, 128, JT * S)
    )

    maskneg = np.where(np.asarray(adj) > 0, np.float32(0), NEG).astype(np.float32)
    I256 = np.eye(256, dtype=np.float32)
    I_tiled = np.ascontiguousarray(
        I256.reshape(2, 128, 256).transpose(1, 0, 2).reshape(128, 512)
    )

    Whs = [np.asarray(Whr, np.float32), np.asarray(Whz, np.float32), np.asarray(Whn, np.float32)]
    ball = np.concatenate([np.asarray(bhr), np.asarray(bhz), np.asarray(bhn)]).astype(np.float32)

    Wo_full = np.asarray(Wo, np.float32).reshape(N)
    Wo_tiled = np.ascontiguousarray(Wo_full.reshape(JT, 128).T)  # [128, 16]
    bo_val = np.asarray(bo, np.float32).reshape(1, 1)

    in_maps = []
    for cid in range(NC):
        isl = slice(cid * ISL, (cid + 1) * ISL)
        Wsl = np.concatenate([Wg.T[:, isl] for Wg in Whs], axis=1)  # [2048, 768]
        W_tiled = np.ascontiguousarray(
            Wsl.reshape(JT, 128, G3).transpose(1, 0, 2).reshape(128, JT * G3)
        )
        mask_tiled = np.ascontiguousarray(
            maskneg[isl, :].reshape(2, 128, N).transpose(1, 0, 2).reshape(128, 2 * N)
        )
        xhs = np.ascontiguousarray(Xh[:, :, isl])  # [B, 65, 256]
        b3 = np.concatenate(
            [ball[isl], ball[N + cid * ISL : N + (cid + 1) * ISL], np.zeros(ISL, np.float32)]
        )
        b3_rep = np.ascontiguousarray(np.broadcast_to(b3, (S, G3)))
        bn_rep = np.ascontiguousarray(
            np.broadcast_to(ball[2 * N + cid * ISL : 2 * N + (cid + 1) * ISL], (B, ISL))
        )
        wt, wot = W_tiled, Wo_tiled
        mt_, it_ = mask_tiled, I_tiled
        if cbf16:
            import ml_dtypes
            wt = W_tiled.astype(ml_dtypes.bfloat16)
            wot = Wo_tiled.astype(ml_dtypes.bfloat16)
        if mbf16:
            import ml_dtypes
            mt_ = mask_tiled.astype(ml_dtypes.bfloat16)
            it_ = I_tiled.astype(ml_dtypes.bfloat16)
        in_maps.append(
            dict(
                xh=Xh, xhs=xhs, xt=xt_tiled, mt=MT,
                mask=mt_, ident=it_, ident16=np.eye(B, dtype=np.float32), w=wt,
                b3=b3_rep, bn=bn_rep, wo=wot, bo=bo_val,
            )
        )
    return in_maps


# ------------------------------------------------------------------ v7 ------
HB = B // 2  # half-batch group size for the D AllReduce


def _host_prep_v7(x, adj, Wq, bq, Wk, bk, Whr, bhr, Whz, bhz, Whn, bhn, Wo, bo, dr=False):
    import ml_dtypes

    BF = ml_dtypes.bfloat16
    F8 = ml_dtypes.float8_e4m3
    DRS = 16.0   # fp8 weight scale
    DRT = 1024.0  # total GEMM scale under dr (weights x16, h x64)
    f64 = np.float64
    x = np.asarray(x, np.float32)

    G = np.asarray(Wq, f64).T @ np.asarray(Wk, f64)
    u = np.asarray(Wq, f64).T @ np.asarray(bk, f64)
    v = np.asarray(Wk, f64).T @ np.asarray(bq, f64)
    c = np.asarray(bq, f64) @ np.asarray(bk, f64)
    M = np.block([[G.T, v[:, None]], [u[None, :], np.array([[c]])]]).astype(np.float32)
    MT = np.ascontiguousarray(M.T)

    ones_row = np.ones((B, 1, N), np.float32)
    Xh = np.concatenate([x, ones_row], axis=1)  # [B, 65, N]
    Xh_bf = np.ascontiguousarray(Xh).astype(BF)

    xT = np.transpose(x, (0, 2, 1))  # [B, N, S]
    xt_bf = np.ascontiguousarray(
        xT.reshape(B, JT, 128, S).transpose(0, 2, 1, 3).reshape(B, 128, JT * S)
    ).astype(BF)

    mask01 = (np.asarray(adj) > 0).astype(np.float32)
    Whs = [np.asarray(Whr, np.float32), np.asarray(Whz, np.float32), np.asarray(Whn, np.float32)]
    ball = np.concatenate([np.asarray(bhr), np.asarray(bhz), np.asarray(bhn)]).astype(np.float32)

    Wo_full = np.asarray(Wo, np.float32).reshape(N)
    Wo_tiled = np.ascontiguousarray(Wo_full.reshape(JT, 128).T).astype(BF)  # [128, 16]
    bo_val = np.asarray(bo, np.float32).reshape(1, 1)

    in_maps = []
    for cid in range(NC):
        isl = slice(cid * ISL, (cid + 1) * ISL)
        Wsl = np.concatenate([Wg.T[:, isl] for Wg in Whs], axis=1)  # [2048, 768]
        W_tiled = np.ascontiguousarray(
            Wsl.reshape(JT, 128, G3).transpose(1, 0, 2).reshape(128, JT * G3)
        ).astype(BF)
        # mask01 for this i-slice, j-tiled: [j=2048, i=256] -> [128, 16*256]
        m_sl = np.ascontiguousarray(mask01[isl, :].T)  # [j, i_local]
        m_tiled = np.ascontiguousarray(
            m_sl.reshape(JT, 128, ISL).transpose(1, 0, 2).reshape(128, JT * ISL)
        ).astype(ml_dtypes.bfloat16)
        xhs = np.ascontiguousarray(Xh[:, :, isl])  # [B, 65, 256] fp32
        b2 = np.concatenate([ball[isl], ball[N + cid * ISL : N + (cid + 1) * ISL]])
        if dr:
            b2 = b2 * DRT
        b2_rep = np.ascontiguousarray(np.broadcast_to(b2, (S, 2 * ISL)))
        bnv = ball[2 * N + cid * ISL : 2 * N + (cid + 1) * ISL]
        if dr:
            bnv = bnv * DRT
        bn_row = np.ascontiguousarray(np.broadcast_to(bnv, (B, ISL)))
        m = dict(
            xh=Xh_bf, xhs=xhs, xt=xt_bf, mt=MT, mask=m_tiled, w=W_tiled,
            b2=b2_rep, bn=bn_row, ones16=np.ones((1, B), np.float32),
            ident16=np.eye(B, dtype=np.float32), ident16b=np.eye(B, dtype=BF), wo=Wo_tiled, bo=bo_val,
        )
        if dr:
            # fp8 DoubleRow packing: pair element j(p, e) = c*256 + 128*e + p
            NCH = N // 256
            W8 = np.zeros((128, NCH, 2, G3), np.float32)
            for c in range(NCH):
                for e in range(2):
                    W8[:, c, e, :] = DRS * Wsl[c * 256 + 128 * e : c * 256 + 128 * e + 128, :]
            m["w8"] = np.ascontiguousarray(W8.reshape(128, NCH * 2 * G3)).astype(F8)
            Wo8 = np.zeros((128, NCH, 2), np.float32)
            for c in range(NCH):
                for e in range(2):
                    Wo8[:, c, e] = DRS * Wo_full[c * 256 + 128 * e : c * 256 + 128 * e + 128]
            m["wo8"] = np.ascontiguousarray(Wo8.reshape(128, NCH * 2)).astype(F8)
            m["id16f8"] = np.eye(B, dtype=F8)
        in_maps.append(m)
    return in_maps


def _kernel_body_v7(tc, d, dr=False):
    nc = tc.nc
    RG = [list(range(NC))]
    BF16 = mybir.dt.bfloat16
    F8 = mybir.dt.float8e4
    DR = mybir.MatmulPerfMode.DoubleRow
    NCH = N // 256
    MM = nc.tensor.matmul

    with ExitStack() as ctx:
        const_pool = ctx.enter_context(tc.tile_pool(name="const", bufs=1))
        dram = ctx.enter_context(tc.tile_pool(name="dramscratch", bufs=1, space="DRAM"))

        # constants needed in phase C (loaded early on the gpsimd queue so the
        # sync queue is free for per-batch input streaming)
        if dr:
            w_sb = const_pool.tile([128, NCH * G3 * 2], F8)
            nc.gpsimd.dma_start(w_sb[:], d["w8"])
            wo_sb = const_pool.tile([128, NCH * 2], F8)
            nc.gpsimd.dma_start(wo_sb[:], d["wo8"])
            id16f8_sb = const_pool.tile([B, B], F8)
            nc.gpsimd.dma_start(id16f8_sb[:], d["id16f8"])
        else:
            w_sb = const_pool.tile([128, JT * G3], BF16)
            nc.gpsimd.dma_start(w_sb[:], d["w"])
            wo_sb = const_pool.tile([128, JT], BF16)
            nc.gpsimd.dma_start(wo_sb[:], d["wo"])
        bo_sb = const_pool.tile([1, 1], FP32)
        nc.gpsimd.dma_start(bo_sb[:], d["bo"])
        bn_sb = const_pool.tile([B, ISL], FP32)
        nc.gpsimd.dma_start(bn_sb[:], d["bn"])
        ones16_sb = const_pool.tile([1, B], FP32)
        nc.gpsimd.dma_start(ones16_sb[:], d["ones16"])
        id16_sb = const_pool.tile([B, B], FP32)
        nc.gpsimd.dma_start(id16_sb[:], d["ident16"])
        id16b_sb = const_pool.tile([B, B], BF16)
        nc.gpsimd.dma_start(id16b_sb[:], d["ident16b"])
        # phase A/B constants
        mask_sb = const_pool.tile([128, JT * ISL], BF16)
        nc.sync.dma_start(mask_sb[:], d["mask"])
        mt_sb = const_pool.tile([SA, SA], FP32)
        nc.sync.dma_start(mt_sb[:], d["mt"])
        b2_sb = const_pool.tile([S, 2 * ISL], FP32)
        nc.sync.dma_start(b2_sb[:], d["b2"])

        agg3_dram = dram.tile([B, S, G3], FP32)

        # ========================= phase A/B =========================
        with ExitStack() as actx:
            xh_pool = actx.enter_context(tc.tile_pool(name="xhp", bufs=2))
            small_pool = actx.enter_context(tc.tile_pool(name="smallp", bufs=2))
            e_pool = actx.enter_context(tc.tile_pool(name="ep", bufs=10))
            s_psum = actx.enter_context(tc.tile_pool(name="spsum", bufs=3, space="PSUM"))
            h_psum = actx.enter_context(tc.tile_pool(name="hpsum", bufs=2, space="PSUM"))
            g_psum = actx.enter_context(tc.tile_pool(name="gpsum", bufs=2, space="PSUM"))
            ar_dram = actx.enter_context(tc.tile_pool(name="ardram", bufs=2, space="DRAM"))

            GROUPS = [range(0, 5), range(5, 10), range(10, 15), range(15, 16)]
            QB = 5  # max batches per pipeline group (tile sizing)

            def scores_group(q):
                dall_sb = small_pool.tile([128, QB * JT], FP32, tag="dall")
                for bl, b in enumerate(GROUPS[q]):
                    xh_sb = xh_pool.tile([SA, N], BF16, tag="xh")
                    nc.sync.dma_start(xh_sb[:], d["xh"][b])
                    xhs_sb = small_pool.tile([SA, ISL], FP32, tag="xhs")
                    nc.sync.dma_start(xhs_sb[:], d["xhs"][b])

                    h_ps = h_psum.tile([SA, ISL], FP32, tag="hps")
                    MM(h_ps[:], mt_sb[:], xhs_sb[:], start=True, stop=True)
                    h_sb = small_pool.tile([SA, ISL], BF16, tag="hsb")
                    nc.scalar.copy(h_sb[:], h_ps[:])

                    e_sb = e_pool.tile([128, JT * ISL], BF16, tag="esb")
                    e_tiles[b] = e_sb
                    for jp in range(JT // 2):
                        s_ps = s_psum.tile([128, 2 * ISL], FP32, tag="sps")
                        for k in range(2):
                            jt = 2 * jp + k
                            MM(
                                s_ps[:, k * ISL : (k + 1) * ISL],
                                xh_sb[:, jt * 128 : (jt + 1) * 128],
                                h_sb[:], start=True, stop=True,
                            )
                        eraw = small_pool.tile([128, 2 * ISL], BF16, tag="eraw")
                        nc.scalar.activation(eraw[:], s_ps[:], AF.Exp)
                        nc.vector.tensor_mul(
                            e_sb[:, jp * 2 * ISL : (jp + 1) * 2 * ISL],
                            eraw[:],
                            mask_sb[:, jp * 2 * ISL : (jp + 1) * 2 * ISL],
                        )
                    nc.vector.tensor_reduce(
                        dall_sb[:, bl * JT : (bl + 1) * JT],
                        e_sb[:].rearrange("p (j i) -> p j i", i=ISL),
                        axis=mybir.AxisListType.X, op=mybir.AluOpType.add,
                    )
                nb = len(GROUPS[q])
                ar_in = ar_dram.tile([128, QB * JT], FP32, tag="arin")
                nc.sync.dma_start(ar_in[:, 0 : nb * JT], dall_sb[:, 0 : nb * JT])
                ar_out = ar_dram.tile([128, QB * JT], FP32, tag="arout")
                nc.gpsimd.collective_compute(
                    "AllReduce", mybir.AluOpType.add, replica_groups=RG,
                    ins=[ar_in.opt()], outs=[ar_out.opt()],
                )
                return ar_out

            def finish_group(q, ar_out):
                nb = len(GROUPS[q])
                df_sb = small_pool.tile([128, QB * JT], FP32, tag="dfsb")
                nc.sync.dma_start(df_sb[:, 0 : nb * JT], ar_out[:, 0 : nb * JT])
                dinv_sb = small_pool.tile([128, QB * JT], FP32, tag="dinv")
                nc.vector.reciprocal(dinv_sb[:, 0 : nb * JT], df_sb[:, 0 : nb * JT])
                if dr:
                    dinv16 = small_pool.tile([128, QB * JT], FP32, tag="dinv16")
                    nc.scalar.mul(dinv16[:], dinv_sb[:], 1024.0)
                    return dinv16
                return dinv_sb

            def agg_group(q, dinv_sb):
                for bl, b in enumerate(GROUPS[q]):
                    xt_sb = small_pool.tile([128, JT * S], BF16, tag="xt")
                    nc.sync.dma_start(xt_sb[:], d["xt"][b])
                    e_sb = e_tiles[b]
                    xd_sb = small_pool.tile([128, JT * S], BF16, tag="xdsb")
                    agg_ps = g_psum.tile([S, ISL], FP32, tag="aggps")
                    nc.vector.tensor_mul(
                        xd_sb[:].rearrange("p (j t) -> p j t", t=S),
                        xt_sb[:].rearrange("p (j t) -> p j t", t=S),
                        dinv_sb[:, bl * JT : (bl + 1) * JT, None].to_broadcast(
                            [128, JT, S]
                        ),
                    )
                    for jt in range(JT):
                        MM(
                            agg_ps[:], xd_sb[:, jt * S : (jt + 1) * S],
                            e_sb[:, jt * ISL : (jt + 1) * ISL],
                            start=(jt == 0), stop=(jt == JT - 1),
                        )
                    agg_sb = small_pool.tile([S, G3], FP32, tag="aggsb")
                    nc.vector.tensor_add(agg_sb[:, 0:ISL], agg_ps[:], b2_sb[:, 0:ISL])
                    nc.vector.tensor_add(
                        agg_sb[:, ISL : 2 * ISL], agg_ps[:], b2_sb[:, ISL : 2 * ISL]
                    )
                    nc.scalar.copy(agg_sb[:, 2 * ISL : G3], agg_ps[:])
                    nc.sync.dma_start(agg3_dram[b], agg_sb[:])

            e_tiles = {}
            prev = None
            for q in range(len(GROUPS)):
                ar_out = scores_group(q)
                if prev is not None:
                    agg_group(prev[0], prev[1])
                dinv_sb = finish_group(q, ar_out)
                prev = (q, dinv_sb)
            agg_group(prev[0], prev[1])

        # ========================= phase C =========================
        # ht layout: [128, c*B ..] with global j-chunk c = 2*rank + cc.
        # AG payload per rank: [128, 2, B] p-major (64B contiguous per row).
        with ExitStack() as cctx:
            ht_pool = cctx.enter_context(tc.tile_pool(name="htp", bufs=2))
            gate_pool = cctx.enter_context(tc.tile_pool(name="gatep", bufs=2))
            aggt_pool = cctx.enter_context(tc.tile_pool(name="aggtp", bufs=3))
            seg_psum = cctx.enter_context(tc.tile_pool(name="segpsum", bufs=2, space="PSUM"))
            t_psum = cctx.enter_context(tc.tile_pool(name="tpsum", bufs=1, space="PSUM"))
            ag_dram = cctx.enter_context(tc.tile_pool(name="agdram", bufs=2, space="DRAM"))

            if dr:
                ht_sb = ht_pool.tile([128, NCH * 2 * B], F8, tag="ht")
                nc.vector.memset(ht_sb[:], 0.0)
                h_sb = gate_pool.tile([B, ISL], BF16, tag="hsl")
                nc.vector.memset(h_sb[:], 0.0)

                aggt_sb = aggt_pool.tile([B, G3], FP32, tag="aggt")
                nc.scalar.dma_start(aggt_sb[:], agg3_dram[:, 0, :])

                wv = w_sb[:].rearrange("p (c e n) -> p c e n", c=NCH, e=2)
                for t in range(S):
                    preA = seg_psum.tile([B, 2 * ISL], FP32, tag="preA")
                    preB = seg_psum.tile([B, ISL], FP32, tag="preB")
                    htv = ht_sb[:].rearrange("p (c e b) -> p c e b", c=NCH, e=2)
                    for c in range(NCH):
                        MM(
                            preA[:], htv[:, c],
                            wv[:, c, :, 0 : 2 * ISL],
                            start=(c == 0), stop=(c == NCH - 1), perf_mode=DR,
                        )
                    for c in range(NCH):
                        MM(
                            preB[:], htv[:, c],
                            wv[:, c, :, 2 * ISL : G3],
                            start=(c == 0), stop=(c == NCH - 1), perf_mode=DR,
                        )
                    if t + 1 < S:
                        aggt_next = aggt_pool.tile([B, G3], FP32, tag="aggt")
                        nc.scalar.dma_start(aggt_next[:], agg3_dram[:, t + 1, :])

                    rzin = gate_pool.tile([B, 2 * ISL], FP32, tag="rzin")
                    nc.vector.tensor_add(rzin[:], preA[:], aggt_sb[:, 0 : 2 * ISL])
                    rz = gate_pool.tile([B, 2 * ISL], FP32, tag="rz")
                    nc.scalar.activation(rz[:], rzin[:], AF.Sigmoid, scale=1.0 / 1024)
                    nt1 = gate_pool.tile([B, ISL], FP32, tag="nt1")
                    nc.vector.tensor_add(nt1[:], preB[:], bn_sb[:])
                    nt2 = gate_pool.tile([B, ISL], FP32, tag="nt2")
                    nc.vector.tensor_mul(nt2[:], nt1[:], rz[:, 0:ISL])
                    nin = gate_pool.tile([B, ISL], FP32, tag="nin")
                    nc.vector.tensor_add(nin[:], nt2[:], aggt_sb[:, 2 * ISL : G3])
                    ng = gate_pool.tile([B, ISL], FP32, tag="ng")
                    nc.scalar.activation(ng[:], nin[:], AF.Tanh, scale=1.0 / 1024)
                    hmn = gate_pool.tile([B, ISL], FP32, tag="hmn")
                    nc.vector.tensor_sub(hmn[:], h_sb[:], ng[:])
                    zh = gate_pool.tile([B, ISL], FP32, tag="zh")
                    nc.vector.tensor_mul(zh[:], rz[:, ISL : 2 * ISL], hmn[:])
                    h_new = gate_pool.tile([B, ISL], BF16, tag="hsl")
                    nc.vector.tensor_add(h_new[:], zh[:], ng[:])
                    h_sb = h_new
                    aggt_sb = aggt_next if t + 1 < S else aggt_sb

                    # transpose halves (bf16) -> [p, (e b)]; fp8 cast in copy
                    tp_ps = t_psum.tile([128, 2 * B], BF16, tag="tpps")
                    for e in range(2):
                        nc.tensor.transpose(
                            tp_ps[:, e * B : (e + 1) * B],
                            h_new[:, e * 128 : (e + 1) * 128], id16b_sb[:],
                        )
                    tp_sb = gate_pool.tile([128, 2 * B], F8, tag="tpsb")
                    nc.scalar.mul(tp_sb[:], tp_ps[:], 64.0)
                    ag_in = ag_dram.tile([128, 2 * B], F8, tag="agin")
                    nc.gpsimd.dma_start(ag_in[:], tp_sb[:])
                    ag_out = ag_dram.tile([NC * 128, 2 * B], F8, tag="agout", addr_space="Shared")
                    nc.gpsimd.collective_compute(
                        "AllGather", mybir.AluOpType.bypass, replica_groups=RG,
                        ins=[ag_in.opt()], outs=[ag_out.opt()],
                    )
                    ht_sb = ht_pool.tile([128, NCH * 2 * B], F8, tag="ht")
                    ag_v = ag_out[:].rearrange("(c p) x -> p c x", p=128)
                    ht_v = ht_sb[:].rearrange("p (c x) -> p c x", c=NCH)
                    nc.sync.dma_start(ht_v[:, 0:1, :], ag_v[:, 0:1, :])
                    nc.scalar.dma_start(ht_v[:, 1:2, :], ag_v[:, 1:2, :])
                    nc.gpsimd.dma_start(ht_v[:, 2:4, :], ag_v[:, 2:4, :])
                    nc.sync.dma_start(ht_v[:, 4:6, :], ag_v[:, 4:6, :])
                    nc.scalar.dma_start(ht_v[:, 6:8, :], ag_v[:, 6:8, :])

                out_ps = t_psum.tile([1, B], FP32, tag="outps")
                for ce in range(2 * NCH):
                    MM(
                        out_ps[:], wo_sb[:, ce : ce + 1],
                        ht_sb[:, ce * B : (ce + 1) * B],
                        start=(ce == 0), stop=(ce == 2 * NCH - 1),
                    )
                out_f = gate_pool.tile([1, B], FP32, tag="outf")
                nc.scalar.mul(out_f[:], out_ps[:], 1.0 / 1024)
                out_sb = gate_pool.tile([1, B], FP32, tag="outsb")
                nc.vector.tensor_scalar_add(out_sb[:], out_f[:], bo_sb[0:1, 0:1])
                nc.sync.dma_start(d["out"], out_sb[:])
                return

            ht_sb = ht_pool.tile([128, JT * B], BF16, tag="ht")
            nc.vector.memset(ht_sb[:], 0.0)
            h_sb = gate_pool.tile([B, ISL], BF16, tag="hsl")
            nc.vector.memset(h_sb[:], 0.0)

            aggt_sb = aggt_pool.tile([B, G3], FP32, tag="aggt")
            nc.scalar.dma_start(aggt_sb[:], agg3_dram[:, 0, :])

            KG = JT // 2  # k-chunks per column-tile group
            for t in range(S):
                # ---- gate GEMM: seg A (r|z cols 0:512), then seg B (n cols
                # 512:768); 2-way PE column tiling (groups stream concurrently)
                preA = seg_psum.tile([128, 2 * ISL], FP32, tag="preA")
                preB = seg_psum.tile([128, ISL], FP32, tag="preB")
                for k in range(KG):
                    for g in range(2):
                        jc = g * KG + k
                        MM(
                            preA[64 * g : 64 * g + B, :],
                            ht_sb[:, jc * B : (jc + 1) * B],
                            w_sb[:, jc * G3 : jc * G3 + 2 * ISL],
                            start=(k == 0), stop=(k == KG - 1),
                            tile_position=(0, 64 * g),
                        )
                for k in range(KG):
                    for g in range(2):
                        jc = g * KG + k
                        MM(
                            preB[64 * g : 64 * g + B, :],
                            ht_sb[:, jc * B : (jc + 1) * B],
                            w_sb[:, jc * G3 + 2 * ISL : (jc + 1) * G3],
                            start=(k == 0), stop=(k == KG - 1),
                            tile_position=(0, 64 * g),
                        )

                # prefetch next aggt (scalar queue)
                if t + 1 < S:
                    aggt_next = aggt_pool.tile([B, G3], FP32, tag="aggt")
                    nc.scalar.dma_start(aggt_next[:], agg3_dram[:, t + 1, :])

                # ---- gates: fp32 where a PSUM operand sets the DVE rate
                # anyway; bf16 on the pure-SBUF chain ops
                rzt = gate_pool.tile([B, 2 * ISL], FP32, tag="rzt")
                nc.vector.tensor_add(rzt[:], preA[64 : 64 + B, :], aggt_sb[:, 0 : 2 * ISL])
                rzin = gate_pool.tile([B, 2 * ISL], FP32, tag="rzin")
                nc.vector.tensor_add(rzin[:], preA[0:B, :], rzt[:])
                rz = gate_pool.tile([B, 2 * ISL], BF16, tag="rz")
                nc.scalar.activation(rz[:], rzin[:], AF.Sigmoid)
                zc = gate_pool.tile([B, ISL], BF16, tag="zc")
                nc.scalar.activation(
                    zc[:], rz[:, ISL : 2 * ISL], AF.Copy, bias=1.0, scale=-1.0
                )

                nt1a = gate_pool.tile([B, ISL], FP32, tag="nt1a")
                nc.vector.tensor_add(nt1a[:], preB[64 : 64 + B, :], bn_sb[:])
                nt1 = gate_pool.tile([B, ISL], FP32, tag="nt1")
                nc.vector.tensor_add(nt1[:], preB[0:B, :], nt1a[:])
                nt2 = gate_pool.tile([B, ISL], BF16, tag="nt2")
                nc.vector.tensor_mul(nt2[:], nt1[:], rz[:, 0:ISL])
                nin = gate_pool.tile([B, ISL], BF16, tag="nin")
                nc.vector.tensor_add(nin[:], nt2[:], aggt_sb[:, 2 * ISL : G3])
                zh2 = gate_pool.tile([B, ISL], BF16, tag="zh2")
                nc.vector.tensor_mul(zh2[:], rz[:, ISL : 2 * ISL], h_sb[:])
                ng = gate_pool.tile([B, ISL], BF16, tag="ng")
                nc.scalar.activation(ng[:], nin[:], AF.Tanh)
                t1 = gate_pool.tile([B, ISL], BF16, tag="t1")
                nc.vector.tensor_mul(t1[:], ng[:], zc[:])
                h_new = gate_pool.tile([B, ISL], BF16, tag="hsl")
                nc.vector.tensor_add(h_new[:], t1[:], zh2[:])
                h_sb = h_new
                aggt_sb = aggt_next if t + 1 < S else aggt_sb

                # ---- transpose h slice -> [128, 2*B], AllGather, reload ----
                tp_ps = t_psum.tile([128, 2 * B], BF16, tag="tpps")
                for cc in range(2):
                    nc.tensor.transpose(
                        tp_ps[:, cc * B : (cc + 1) * B],
                        h_new[:, cc * 128 : (cc + 1) * 128], id16b_sb[:],
                    )
                tp_sb = gate_pool.tile([128, 2 * B], BF16, tag="tpsb")
                nc.scalar.copy(tp_sb[:], tp_ps[:])
                ag_in = ag_dram.tile([2 * 128, B], BF16, tag="agin")
                ag_in_v = ag_in[:].rearrange("(p c) b -> p (c b)", c=2)
                nc.gpsimd.dma_start(ag_in_v[0:64, :], tp_sb[0:64, :])
                nc.sync.dma_start(ag_in_v[64:128, :], tp_sb[64:128, :])
                ag_out = ag_dram.tile([N, B], BF16, tag="agout", addr_space="Shared")
                nc.gpsimd.collective_compute(
                    "AllGather", mybir.AluOpType.bypass, replica_groups=RG,
                    ins=[ag_in.opt()], outs=[ag_out.opt()],
                )
                ht_sb = ht_pool.tile([128, JT * B], BF16, tag="ht")
                # ag_out rows: r*256 + p*2 + cc ; SBUF chunk c = 2r + cc
                ag_v = ag_out[:].rearrange("(r p c) b -> p r (c b)", p=128, c=2)
                ht_v = ht_sb[:].rearrange("p (r cb) -> p r cb", r=NC)
                nc.sync.dma_start(ht_v[:, 0:8:4, :], ag_v[:, 0:8:4, :])
                nc.scalar.dma_start(ht_v[:, 1:8:4, :], ag_v[:, 1:8:4, :])
                nc.gpsimd.dma_start(ht_v[:, 2:8:4, :], ag_v[:, 2:8:4, :])
                nc.sync.dma_start(ht_v[:, 3:8:4, :], ag_v[:, 3:8:4, :])

            # output head
            out_ps = t_psum.tile([1, B], FP32, tag="outps")
            for jc in range(JT):
                MM(
                    out_ps[:], wo_sb[:, jc : jc + 1], ht_sb[:, jc * B : (jc + 1) * B],
                    start=(jc == 0), stop=(jc == JT - 1),
                )
            out_sb = gate_pool.tile([1, B], FP32, tag="outsb")
            nc.vector.tensor_scalar_add(out_sb[:], out_ps[:], bo_sb[0:1, 0:1])
            nc.sync.dma_start(d["out"], out_sb[:])


def _build_v7(dr=False):
    nc = bacc.Bacc("TRN2", target_bir_lowering=False, debug=False, num_devices=NC)
    BF16 = mybir.dt.bfloat16
    F8 = mybir.dt.float8e4
    d = dict(
        xh=nc.dram_tensor("xh", [B, SA, N], BF16, kind="ExternalInput").ap(),
        xhs=nc.dram_tensor("xhs", [B, SA, ISL], FP32, kind="ExternalInput").ap(),
        xt=nc.dram_tensor("xt", [B, 128, JT * S], BF16, kind="ExternalInput").ap(),
        mt=nc.dram_tensor("mt", [SA, SA], FP32, kind="ExternalInput").ap(),
        mask=nc.dram_tensor("mask", [128, JT * ISL], BF16, kind="ExternalInput").ap(),
        w=nc.dram_tensor("w", [128, JT * G3], BF16, kind="ExternalInput").ap(),
        b2=nc.dram_tensor("b2", [S, 2 * ISL], FP32, kind="ExternalInput").ap(),
        bn=nc.dram_tensor("bn", [B, ISL], FP32, kind="ExternalInput").ap(),
        ones16=nc.dram_tensor("ones16", [1, B], FP32, kind="ExternalInput").ap(),
        ident16=nc.dram_tensor("ident16", [B, B], FP32, kind="ExternalInput").ap(),
        ident16b=nc.dram_tensor("ident16b", [B, B], BF16, kind="ExternalInput").ap(),
        wo=nc.dram_tensor("wo", [128, JT], BF16, kind="ExternalInput").ap(),
        bo=nc.dram_tensor("bo", [1, 1], FP32, kind="ExternalInput").ap(),
        out=nc.dram_tensor("out", [1, B], FP32, kind="ExternalOutput").ap(),
    )
    if dr:
        d["w8"] = nc.dram_tensor("w8", [128, (N // 256) * G3 * 2], F8, kind="ExternalInput").ap()
        d["wo8"] = nc.dram_tensor("wo8", [128, (N // 256) * 2], F8, kind="ExternalInput").ap()
        d["id16f8"] = nc.dram_tensor("id16f8", [B, B], F8, kind="ExternalInput").ap()
    with tile.TileContext(nc) as tc:
        _kernel_body_v7(tc, d, dr=dr)
    nc.compile()
    return nc


# ------------------------------------------------------------------ kernel IR
def _kernel_body(tc, d, variant="abc"):
    nc = tc.nc
    RG = [list(range(NC))]
    # variant features
    f_ab_only = variant == "ab"
    f_ag = variant not in ("noag", "cgemm", "cgates", "o5na")  # real collective AG
    f_warm = variant in ("o5warm",)  # dummy PE work during AG gap (HAM warmth)
    f_gemm = variant != "cgates"
    f_gates = variant not in ("cgemm",)
    f_opt = variant == "opt1"  # (legacy serial col-tiling bundle)
    f_ct = variant in ("opt6",)   # interleaved 2-way column-tiled GEMM
    f_ash = variant in ("opt6",)  # AllGather output in Shared scratchpad
    f_sdma = variant in ("opt6",) # split hT reload DMA
    f_mc = variant in ("opt6",)   # merged transpose copy
    f_r = variant in ("opt2",)  # float32r matmul operands (4x PE rate)
    OPT5F = ("opt3", "opt4", "opt5", "opt6", "o5na", "o5warm")
    f_cbf = variant in OPT5F  # phase-C GEMM + h in bf16
    f_mbf = variant in OPT5F[1:]  # mask/identity matmuls in bf16 (exact)
    f_abf = variant in ("opt5", "opt6", "o5na", "o5warm")  # AGG matmul operands in bf16
    f_ar = variant in ("opt4",)   # AGG matmul operands in float32r (HW-broken)
    f_sr = variant in ("opt4",)   # scores matmul operands in float32r (HW-broken)
    BF16 = mybir.dt.bfloat16
    F32R = mybir.dt.float32r
    CDT = BF16 if f_cbf else FP32
    MDT = BF16 if f_mbf else FP32

    def MM(out, lhsT, rhs, **kw):
        if f_r:
            lhsT = lhsT.bitcast(mybir.dt.float32r)
            rhs = rhs.bitcast(mybir.dt.float32r)
        nc.tensor.matmul(out, lhsT, rhs, **kw)

    if variant == "nop":
        # timing calibration: minimal kernel (one tiny DMA in/out)
        with tc.tile_pool(name="nopp", bufs=1) as nopp:
            fo = nopp.tile([1, B], FP32)
            nc.sync.dma_start(fo[:], d["bo"][0:1, 0:1].to_broadcast([1, B]))
            nc.sync.dma_start(d["out"], fo[:])
        return

    with ExitStack() as ctx:
        const_pool = ctx.enter_context(tc.tile_pool(name="const", bufs=1))
        dram = ctx.enter_context(tc.tile_pool(name="dramscratch", bufs=1, space="DRAM"))

        mask_sb = const_pool.tile([128, 2 * N], MDT)
        nc.sync.dma_start(mask_sb[:], d["mask"])
        id_sb = const_pool.tile([128, 512], MDT)
        nc.sync.dma_start(id_sb[:], d["ident"])
        id16_sb = const_pool.tile([B, B], FP32)
        nc.sync.dma_start(id16_sb[:], d["ident16"])
        mt_sb = const_pool.tile([SA, SA], FP32)
        nc.sync.dma_start(mt_sb[:], d["mt"])
        b3_sb = const_pool.tile([S, G3], FP32)
        nc.sync.dma_start(b3_sb[:], d["b3"])
        bn_sb = const_pool.tile([B, ISL], FP32)
        nc.sync.dma_start(bn_sb[:], d["bn"])
        wo_sb = const_pool.tile([128, JT], CDT)
        nc.sync.dma_start(wo_sb[:], d["wo"])
        bo_sb = const_pool.tile([1, 1], FP32)
        nc.sync.dma_start(bo_sb[:], d["bo"])
        w_sb = const_pool.tile([128, JT * G3], CDT)
        nc.sync.dma_start(w_sb[:], d["w"])

        agg3_dram = dram.tile([B, S, G3], FP32)

        # ========================= phase A/B =========================
        with ExitStack() as actx:
            xh_pool = actx.enter_context(tc.tile_pool(name="xhp", bufs=2))
            small_pool = actx.enter_context(tc.tile_pool(name="smallp", bufs=2))
            e_pool = actx.enter_context(tc.tile_pool(name="ep", bufs=2))
            s_psum = actx.enter_context(tc.tile_pool(name="spsum", bufs=3, space="PSUM"))
            h_psum = actx.enter_context(tc.tile_pool(name="hpsum", bufs=2, space="PSUM"))
            g_psum = actx.enter_context(tc.tile_pool(name="gpsum", bufs=2, space="PSUM"))
            ar_dram = actx.enter_context(tc.tile_pool(name="ardram", bufs=2, space="DRAM"))

            for b in range(B):
                xh_sb = xh_pool.tile([SA, N], FP32, tag="xh")
                nc.sync.dma_start(xh_sb[:], d["xh"][b])
                xhs_sb = small_pool.tile([SA, ISL], FP32, tag="xhs")
                nc.sync.dma_start(xhs_sb[:], d["xhs"][b])
                xt_sb = small_pool.tile([128, JT * S], FP32, tag="xt")
                nc.sync.dma_start(xt_sb[:], d["xt"][b])

                # H = M @ Xh[:, islice]  -> [65, 256]
                h_ps = h_psum.tile([SA, ISL], FP32, tag="hps")
                MM(h_ps[:], mt_sb[:], xhs_sb[:], start=True, stop=True)
                h_sb = small_pool.tile([SA, ISL], F32R if f_sr else FP32, tag="hsb")
                nc.scalar.copy(h_sb[:], h_ps[:])
                if f_sr:
                    xh_r = xh_pool.tile([SA, N], F32R, tag="xhr")
                    nc.scalar.copy(xh_r[:], xh_sb[:])
                else:
                    xh_r = xh_sb

                # E tiles: e_sb[p, jt*256 + i] = exp(s[i, jt*128+p] + maskneg)
                e_sb = e_pool.tile(
                    [128, JT * ISL], BF16 if f_abf else (F32R if f_ar else FP32), tag="esb"
                )
                for jt in range(JT):
                    s_ps = s_psum.tile([128, ISL], FP32, tag="sps")
                    MM(
                        s_ps[:], mask_sb[:, jt * 128 : (jt + 1) * 128],
                        id_sb[:, 0:ISL], start=True, stop=False,
                    )
                    MM(
                        s_ps[:], mask_sb[:, N + jt * 128 : N + (jt + 1) * 128],
                        id_sb[:, ISL : 2 * ISL], start=False, stop=False,
                    )
                    nc.tensor.matmul(
                        s_ps[:], xh_r[:, jt * 128 : (jt + 1) * 128],
                        h_sb[:], start=False, stop=True,
                    )
                    nc.scalar.activation(
                        e_sb[:, jt * ISL : (jt + 1) * ISL], s_ps[:], AF.Exp
                    )

                # D partial = sum_i E  (one 3D reduce)
                d_sb = small_pool.tile([128, JT], FP32, tag="dsb")
                e_red = e_sb[:].bitcast(FP32) if f_ar else e_sb[:]
                nc.vector.tensor_reduce(
                    d_sb[:], e_red.rearrange("p (j i) -> p j i", i=ISL),
                    axis=mybir.AxisListType.X, op=mybir.AluOpType.add,
                )
                ar_in = ar_dram.tile([128, JT], FP32, tag="arin")
                nc.sync.dma_start(ar_in[:], d_sb[:])
                ar_out = ar_dram.tile([128, JT], FP32, tag="arout")
                nc.gpsimd.collective_compute(
                    "AllReduce", mybir.AluOpType.add, replica_groups=RG,
                    ins=[ar_in.opt()], outs=[ar_out.opt()],
                )
                df_sb = small_pool.tile([128, JT], FP32, tag="dfsb")
                nc.sync.dma_start(df_sb[:], ar_out[:])
                dinv_sb = small_pool.tile([128, JT], FP32, tag="dinv")
                nc.vector.reciprocal(dinv_sb[:], df_sb[:])

                # AGG[t, i] = sum_j (xT[j,t] * Dinv[j]) E[j, i]
                xd_sb = small_pool.tile(
                    [128, JT * S], BF16 if f_abf else (F32R if f_ar else FP32), tag="xdsb"
                )
                agg_ps = g_psum.tile([S, ISL], FP32, tag="aggps")
                for jt in range(JT):
                    nc.vector.tensor_scalar_mul(
                        xd_sb[:, jt * S : (jt + 1) * S],
                        xt_sb[:, jt * S : (jt + 1) * S],
                        dinv_sb[:, jt : jt + 1],
                    )
                    MM(
                        agg_ps[:], xd_sb[:, jt * S : (jt + 1) * S],
                        e_sb[:, jt * ISL : (jt + 1) * ISL],
                        start=(jt == 0), stop=(jt == JT - 1),
                    )

                # agg3 = [agg + bhr | agg + bhz | agg] -> DRAM[b]
                agg_sb = small_pool.tile([S, G3], FP32, tag="aggsb")
                nc.vector.tensor_add(agg_sb[:, 0:ISL], agg_ps[:], b3_sb[:, 0:ISL])
                nc.vector.tensor_add(
                    agg_sb[:, ISL : 2 * ISL], agg_ps[:], b3_sb[:, ISL : 2 * ISL]
                )
                nc.scalar.copy(agg_sb[:, 2 * ISL : G3], agg_ps[:])
                nc.sync.dma_start(agg3_dram[b], agg_sb[:])

        if f_ab_only:
            # timing variant: stop after phase A/B; emit a tiny output read
            with tc.tile_pool(name="fin", bufs=1) as fin:
                fo = fin.tile([1, B], FP32)
                nc.sync.dma_start(fo[:], agg3_dram[0, 0:1, 0:B])
                nc.sync.dma_start(d["out"], fo[:])
            return

        # ========================= phase C =========================
        with ExitStack() as cctx:
            ht_pool = cctx.enter_context(tc.tile_pool(name="htp", bufs=2))
            gate_pool = cctx.enter_context(tc.tile_pool(name="gatep", bufs=2))
            aggt_pool = cctx.enter_context(tc.tile_pool(name="aggtp", bufs=3))
            c_psum = cctx.enter_context(tc.tile_pool(name="cpsum", bufs=2, space="PSUM"))
            t_psum = cctx.enter_context(tc.tile_pool(name="tpsum", bufs=1, space="PSUM"))
            ag_dram = cctx.enter_context(tc.tile_pool(name="agdram", bufs=2, space="DRAM"))

            ht_sb = ht_pool.tile([128, JT * B], CDT, tag="ht")
            nc.vector.memset(ht_sb[:], 0.0)
            h_sb = gate_pool.tile([B, ISL], FP32, tag="hsl")
            nc.vector.memset(h_sb[:], 0.0)

            aggt_sb = aggt_pool.tile([B, G3], FP32, tag="aggt")
            nc.sync.dma_start(aggt_sb[:], agg3_dram[:, 0, :])

            if not f_gemm:
                pre_fix = c_psum.tile([B, G3], FP32, tag="prefix")
                nc.vector.memset(pre_fix[:], 0.0)

            for t in range(S):
                # ---- gate GEMM ----
                if f_gemm and f_ct:
                    # interleaved 2-way column tiling: adjacent MMs alternate
                    # PE column groups so they stream concurrently
                    pre_ps = c_psum.tile([48, G3], FP32, tag="preps")
                    for k in range(8):
                        for seg0, seg1 in ((0, 512), (512, G3)):
                            for grp in (0, 1):
                                jc = grp * 8 + k
                                rows = pre_ps[32 * grp : 32 * grp + B, :]
                                lhsT = ht_sb[:, jc * B : (jc + 1) * B]
                                MM(
                                    rows[:, seg0:seg1], lhsT,
                                    w_sb[:, jc * G3 + seg0 : jc * G3 + seg1],
                                    start=(k == 0), stop=(k == 7),
                                    tile_position=(0, 32 * grp),
                                )
                elif f_gemm and f_opt:
                    # 2-way PE column-tiling: j-chunks 0-7 -> col group 0
                    # (psum rows 0:16), chunks 8-15 -> col group 1 (rows 32:48)
                    pre_ps = c_psum.tile([48, G3], FP32, tag="preps")
                    for jc in range(JT):
                        grp = jc // 8
                        rows = pre_ps[32 * grp : 32 * grp + B, :]
                        lhsT = ht_sb[:, jc * B : (jc + 1) * B]
                        MM(
                            rows[:, 0:512], lhsT, w_sb[:, jc * G3 : jc * G3 + 512],
                            start=(jc % 8 == 0), stop=(jc % 8 == 7),
                            tile_position=(0, 32 * grp),
                        )
                        MM(
                            rows[:, 512:G3], lhsT, w_sb[:, jc * G3 + 512 : (jc + 1) * G3],
                            start=(jc % 8 == 0), stop=(jc % 8 == 7),
                            tile_position=(0, 32 * grp),
                        )
                elif f_gemm:
                    pre_ps = c_psum.tile([B, G3], FP32, tag="preps")
                    for jc in range(JT):
                        lhsT = ht_sb[:, jc * B : (jc + 1) * B]
                        MM(
                            pre_ps[:, 0:512], lhsT, w_sb[:, jc * G3 : jc * G3 + 512],
                            start=(jc == 0), stop=(jc == JT - 1),
                        )
                        MM(
                            pre_ps[:, 512:G3], lhsT, w_sb[:, jc * G3 + 512 : (jc + 1) * G3],
                            start=(jc == 0), stop=(jc == JT - 1),
                        )
                else:
                    pre_ps = pre_fix

                # prefetch next agg (off critical path, SWDGE queue)
                if t + 1 < S:
                    aggt_next = aggt_pool.tile([B, G3], FP32, tag="aggt")
                    nc.gpsimd.dma_start(aggt_next[:], agg3_dram[:, t + 1, :])

                # ---- gates ----
                if f_gates:
                    rzin = gate_pool.tile([B, 2 * ISL], FP32, tag="rzin")
                    if f_opt or f_ct:
                        # merge col-group partials inside the adds (one PSUM
                        # operand per DVE op)
                        rzt = gate_pool.tile([B, 2 * ISL], FP32, tag="rzt")
                        nc.vector.tensor_add(
                            rzt[:], pre_ps[32 : 32 + B, 0 : 2 * ISL], aggt_sb[:, 0 : 2 * ISL]
                        )
                        nc.vector.tensor_add(rzin[:], pre_ps[0:B, 0 : 2 * ISL], rzt[:])
                    else:
                        nc.vector.tensor_add(
                            rzin[:], pre_ps[0:B, 0 : 2 * ISL], aggt_sb[:, 0 : 2 * ISL]
                        )
                    rz = gate_pool.tile([B, 2 * ISL], FP32, tag="rz")
                    nc.scalar.activation(rz[:], rzin[:], AF.Sigmoid)
                    nt1 = gate_pool.tile([B, ISL], FP32, tag="nt1")
                    if f_opt or f_ct:
                        nt1a = gate_pool.tile([B, ISL], FP32, tag="nt1a")
                        nc.vector.tensor_add(
                            nt1a[:], pre_ps[32 : 32 + B, 2 * ISL : G3], bn_sb[:]
                        )
                        nc.vector.tensor_add(nt1[:], pre_ps[0:B, 2 * ISL : G3], nt1a[:])
                    else:
                        nc.vector.tensor_add(nt1[:], pre_ps[0:B, 2 * ISL : G3], bn_sb[:])
                    nt2 = gate_pool.tile([B, ISL], FP32, tag="nt2")
                    nc.vector.tensor_mul(nt2[:], nt1[:], rz[:, 0:ISL])
                    nin = gate_pool.tile([B, ISL], FP32, tag="nin")
                    nc.vector.tensor_add(nin[:], nt2[:], aggt_sb[:, 2 * ISL : G3])
                    ng = gate_pool.tile([B, ISL], FP32, tag="ng")
                    nc.scalar.activation(ng[:], nin[:], AF.Tanh)
                    hmn = gate_pool.tile([B, ISL], FP32, tag="hmn")
                    nc.vector.tensor_sub(hmn[:], h_sb[:], ng[:])
                    zh = gate_pool.tile([B, ISL], FP32, tag="zh")
                    nc.vector.tensor_mul(zh[:], rz[:, ISL : 2 * ISL], hmn[:])
                    h_new = gate_pool.tile([B, ISL], FP32, tag="hsl")
                    nc.vector.tensor_add(h_new[:], zh[:], ng[:])
                else:
                    # timing variant: single bounded op stands in for the gates
                    h_new = gate_pool.tile([B, ISL], FP32, tag="hsl")
                    nc.scalar.activation(h_new[:], pre_ps[0:B, 0:ISL], AF.Tanh)
                h_sb = h_new
                aggt_sb = aggt_next if t + 1 < S else aggt_sb

                # ---- transpose h slice -> [128, 16] x2, AllGather, reload hT ----
                tp_sb = gate_pool.tile([128, 2 * B], CDT, tag="tpsb")
                if f_opt or f_mc:
                    tp_ps = t_psum.tile([128, 2 * B], BF16, tag="tpps")
                    for cch in range(2):
                        nc.tensor.transpose(
                            tp_ps[:, cch * B : (cch + 1) * B],
                            h_new[:, cch * 128 : (cch + 1) * 128], id16_sb[:],
                        )
                    nc.scalar.mul(tp_sb[:], tp_ps[:], 64.0)
                else:
                    for cch in range(2):
                        tp_ps = t_psum.tile([128, B], FP32, tag="tpps")
                        nc.tensor.transpose(
                            tp_ps[:], h_new[:, cch * 128 : (cch + 1) * 128], id16_sb[:]
                        )
                        nc.scalar.copy(tp_sb[:, cch * B : (cch + 1) * B], tp_ps[:])
                if f_warm:
                    # keep the PE HAM clock warm through the AllGather gap:
                    # chained junk matmuls gated on tp_sb (i.e. after the
                    # transposes) accumulating into a scratch PSUM bank
                    warm_ps = t_psum.tile([B, 512], FP32, tag="warmps")
                    for wi in range(16):
                        nc.tensor.matmul(
                            warm_ps[:], tp_sb[:, 0:B],
                            w_sb[:, (wi % JT) * G3 : (wi % JT) * G3 + 512],
                            start=(wi == 0), stop=(wi == 15),
                        )
                ag_in = ag_dram.tile([2 * 128, B], CDT, tag="agin")
                nc.sync.dma_start(
                    ag_in[:].rearrange("(c p) b -> p c b", p=128),
                    tp_sb[:].rearrange("p (c b) -> p c b", c=2),
                )
                ag_out = ag_dram.tile(
                    [N, B], CDT, tag="agout",
                    addr_space=("Shared" if (f_opt or f_ash) else "Local"),
                )
                if f_ag:
                    nc.gpsimd.collective_compute(
                        "AllGather", mybir.AluOpType.bypass, replica_groups=RG,
                        ins=[ag_in.opt()], outs=[ag_out.opt()],
                    )
                else:
                    # timing variant: local DRAM->DRAM copy of per-rank size
                    nc.sync.dma_start(ag_out[0 : 2 * 128, :], ag_in[:])
                ht_sb = ht_pool.tile([128, JT * B], CDT, tag="ht")
                if f_opt or f_sdma:
                    # split reload so the first GEMM chunks can start earlier
                    for half in range(2):
                        nc.sync.dma_start(
                            ht_sb[:, half * 8 * B : (half + 1) * 8 * B].rearrange(
                                "p (c b) -> p c b", c=8
                            ),
                            ag_out[half * 1024 : (half + 1) * 1024, :].rearrange(
                                "(c p) b -> p c b", p=128
                            ),
                        )
                else:
                    nc.sync.dma_start(
                        ht_sb[:].rearrange("p (c b) -> p c b", c=JT),
                        ag_out[:].rearrange("(c p) b -> p c b", p=128),
                    )

            # output head: out[b] = sum_j h[b, j] Wo[j] + bo  (full h from last AG)
            out_ps = t_psum.tile([1, B], FP32, tag="outps")
            for jc in range(JT):
                nc.tensor.matmul(
                    out_ps[:], wo_sb[:, jc : jc + 1], ht_sb[:, jc * B : (jc + 1) * B],
                    start=(jc == 0), stop=(jc == JT - 1),
                )
            out_sb = gate_pool.tile([1, B], FP32, tag="outsb")
            nc.vector.tensor_scalar_add(out_sb[:], out_ps[:], bo_sb[0:1, 0:1])
            nc.sync.dma_start(d["out"], out_sb[:])


def _build(variant="abc"):
    nc = bacc.Bacc("TRN2", target_bir_lowering=False, debug=False, num_devices=NC)
    CDT = mybir.dt.bfloat16 if variant in ("opt3", "opt4", "opt5", "opt6", "o5na", "o5warm") else FP32
    MDT = mybir.dt.bfloat16 if variant in ("opt4", "opt5", "opt6", "o5na", "o5warm") else FP32
    d = dict(
        xh=nc.dram_tensor("xh", [B, SA, N], FP32, kind="ExternalInput").ap(),
        xhs=nc.dram_tensor("xhs", [B, SA, ISL], FP32, kind="ExternalInput").ap(),
        xt=nc.dram_tensor("xt", [B, 128, JT * S], FP32, kind="ExternalInput").ap(),
        mt=nc.dram_tensor("mt", [SA, SA], FP32, kind="ExternalInput").ap(),
        mask=nc.dram_tensor("mask", [128, 2 * N], MDT, kind="ExternalInput").ap(),
        ident=nc.dram_tensor("ident", [128, 512], MDT, kind="ExternalInput").ap(),
        ident16=nc.dram_tensor("ident16", [B, B], FP32, kind="ExternalInput").ap(),
        w=nc.dram_tensor("w", [128, JT * G3], CDT, kind="ExternalInput").ap(),
        b3=nc.dram_tensor("b3", [S, G3], FP32, kind="ExternalInput").ap(),
        bn=nc.dram_tensor("bn", [B, ISL], FP32, kind="ExternalInput").ap(),
        wo=nc.dram_tensor("wo", [128, JT], CDT, kind="ExternalInput").ap(),
        bo=nc.dram_tensor("bo", [1, 1], FP32, kind="ExternalInput").ap(),
        out=nc.dram_tensor("out", [1, B], FP32, kind="ExternalOutput").ap(),
    )
    with tile.TileContext(nc) as tc:
        _kernel_body(tc, d, variant=variant)
    nc.compile()  # bacc register allocation / DCE / fusion
    return nc


def prep_and_build(inputs, variant="v7"):
    if variant == "v7":
        return _host_prep_v7(**inputs), _build_v7()
    if variant == "v8":
        return _host_prep_v7(**inputs, dr=True), _build_v7(dr=True)
    cbf = variant in ("opt3", "opt4", "opt5", "opt6", "o5na", "o5warm")
    mbf = variant in ("opt4", "opt5", "opt6", "o5na", "o5warm")
    in_maps = _host_prep(**inputs, cbf16=cbf, mbf16=mbf)
    nc = _build(variant)
    return in_maps, nc


def run_with_results(inputs, trace=False, variant="abc", **kw):
    in_maps, nc = prep_and_build(inputs, variant)
    res = run_bass_kernel_spmd(
        nc, in_maps, core_ids=list(range(NC)), trace=trace, **kw
    )
    out = np.asarray(res.results[0]["out"], np.float32).reshape(B)
    return out, res


def kernel(**inputs) -> np.ndarray:
    out, _ = run_with_results(inputs, variant="v7")
    return out


if __name__ == "__main__":
    import reference

    inputs = {k: np.asarray(v) for k, v in reference.setup_inputs().items()}
    out = kernel(**inputs)
    print("kernel out:", out)

